# revision 22
# baseline (speedup 1.0000x reference)
"""Trainium2 Bass kernel for nn_ACWAN (embedding + GRU + cosine-attention +
gated-state recurrence + output projection), data-parallel over batch on 8
NeuronCores.

Self-contained: hardcodes all shapes; host side only reformats weights/indices
and gathers per-core outputs.
"""
import sys

sys.path.insert(0, "/opt/trn_rl_repo")

import numpy as np
import ml_dtypes

import concourse.bass as bass
import concourse.mybir as mybir
from concourse.tile import TileContext
from concourse.bass_utils import run_bass_kernel_spmd
from concourse.masks import make_identity

# ---- problem dims (hardcoded) ----
B, T, E, H, NT, NCLS = 256, 512, 200, 100, 100000, 5
NCORES = 8
BS = B // NCORES            # 32 batch rows per core
TCH = 32                    # timesteps per chunk
NCH = T // TCH              # 16 chunks
SLOTS = BS * T              # 16384 gather slots per core (k = t*BS + b)
GT = SLOTS // 128           # 128 gather tiles (128 rows each)
GPC = GT // NCH             # 8 gather tiles per chunk

F32 = mybir.dt.float32
BF16 = mybir.dt.bfloat16
I32 = mybir.dt.int32
BF = ml_dtypes.bfloat16
AF = mybir.ActivationFunctionType
OP = mybir.AluOpType

MAX_WAITS_DEFAULT = 1


def _split_excess_waits(nc):
    """walrus here accepts very few sem-waits per instruction; hoist extras
    onto NoOps (1 wait each) placed just before, on the same engine."""
    n_fix = 0
    for f in nc.m.functions:
        for bb in f.blocks:
            out = []
            changed = False
            for ins in bb.instructions:
                si = ins.sync_info
                limit = MAX_WAITS_DEFAULT
                if si is not None and si.on_wait and len(si.on_wait) > limit:
                    waits = list(si.on_wait)
                    extra, keep = waits[:-limit], waits[-limit:]
                    for k, w in enumerate(extra):
                        out.append(
                            mybir.InstNoOp(
                                name=f"{ins.name}-wsplit{k}",
                                sync_info=mybir.SyncInfo(on_wait=[w], on_update=[]),
                                bass_nofuse=True,
                                engine=ins.engine,
                            )
                        )
                    ins.sync_info = mybir.SyncInfo(
                        on_wait=keep, on_update=list(si.on_update)
                    )
                    n_fix += 1
                    changed = True
                out.append(ins)
            if changed:
                bb.instructions = out
    return n_fix


def build_graph():
    nc = bass.Bass()

    dp = nc.declare_dram_parameter
    emb = dp("emb", [NT, E], F32, isOutput=False)
    idx = dp("idx", [128, GT], I32, isOutput=False)          # [p, tile]
    wih = dp("wih", [100, 600], BF16, isOutput=False)        # [e, (chunk,gate,m)]
    whh = dp("whh", [100, 300], BF16, isOutput=False)        # [h, (gate,m)]
    biases = dp("biases", [100, 3], F32, isOutput=False)     # r,z fused; n = b_ih_n
    bhhn = dp("bhhn", [100, 1], F32, isOutput=False)         # b_hh n-gate column
    wti = dp("wti", [100, 100], BF16, isOutput=False)
    bti = dp("bti", [100, 1], F32, isOutput=False)           # b_ti + b_ts fused
    wls = dp("wls", [100, 100], BF16, isOutput=False)
    wli = dp("wli", [100, 1], BF16, isOutput=False)
    blgr = dp("blgr", [1, 1], F32, isOutput=False)
    wts = dp("wts", [100, 100], BF16, isOutput=False)
    wout = dp("wout", [100, NCLS], F32, isOutput=False)
    bout = dp("bout", [NCLS, 1], F32, isOutput=False)
    lens = dp("lens", [1, BS], F32, isOutput=False)
    tvals = dp("tvals", [8, 128], F32, isOutput=False)       # p*4 + f//32
    out_d = dp("out", [NCLS, BS], F32, isOutput=True)

    with TileContext(nc) as tc:
        with tc.tile_pool(name="const", bufs=1) as cp, \
             tc.tile_pool(name="big", bufs=1) as bigp, \
             tc.tile_pool(name="xpring", bufs=4) as xpp, \
             tc.tile_pool(name="rnnring", bufs=3) as rnp, \
             tc.tile_pool(name="zbring", bufs=3) as zbp, \
             tc.tile_pool(name="gring", bufs=6) as gp, \
             tc.tile_pool(name="rtring", bufs=3) as rtp, \
             tc.tile_pool(name="s1sc", bufs=3) as s1p, \
             tc.tile_pool(name="s2sc", bufs=3) as s2p, \
             tc.tile_pool(name="blksc", bufs=1) as bkp, \
             tc.tile_pool(name="pa", bufs=2, space="PSUM") as pa, \
             tc.tile_pool(name="p1", bufs=2, space="PSUM") as p1, \
             tc.tile_pool(name="pg", bufs=2, space="PSUM") as pg, \
             tc.tile_pool(name="pts", bufs=2, space="PSUM") as pts:

            # ---------- load constants ----------
            idx_sb = cp.tile([128, GT], I32)
            nc.sync.dma_start(out=idx_sb[:], in_=idx[:])
            wih_sb = cp.tile([100, 600], BF16)
            nc.sync.dma_start(out=wih_sb[:], in_=wih[:])
            whh_sb = cp.tile([100, 300], BF16)
            nc.sync.dma_start(out=whh_sb[:], in_=whh[:])
            bias_sb = cp.tile([100, 3], F32)
            nc.sync.dma_start(out=bias_sb[:], in_=biases[:])
            bhhn_sb = cp.tile([100, 1], F32)
            nc.sync.dma_start(out=bhhn_sb[:], in_=bhhn[:])
            wti_sb = cp.tile([100, 100], BF16)
            nc.sync.dma_start(out=wti_sb[:], in_=wti[:])
            bti_sb = cp.tile([100, 1], F32)
            nc.sync.dma_start(out=bti_sb[:], in_=bti[:])
            wlsr_sb = cp.tile([100, 100], BF16)
            nc.sync.dma_start(out=wlsr_sb[:], in_=wls[:])
            wli_sb = cp.tile([100, 1], BF16)
            nc.sync.dma_start(out=wli_sb[:], in_=wli[:])
            blgr_sb = cp.tile([1, 1], F32)
            nc.sync.dma_start(out=blgr_sb[:], in_=blgr[:])
            wts_sb = cp.tile([100, 100], BF16)
            nc.sync.dma_start(out=wts_sb[:], in_=wts[:])
            wout_sb = cp.tile([100, NCLS], F32)
            nc.sync.dma_start(out=wout_sb[:], in_=wout[:])
            bout_sb = cp.tile([NCLS, 1], F32)
            nc.sync.dma_start(out=bout_sb[:], in_=bout[:])
            tvals_sb = cp.tile([8, 128], F32)
            nc.sync.dma_start(out=tvals_sb[:], in_=tvals[:])

            lens_sb = cp.tile([8, 128], F32)
            lens_bcast = bass.AP(
                tensor=lens[:].tensor, offset=0,
                ap=[[0, 8], [0, 4], [1, BS]],
            )
            nc.sync.dma_start(
                out=lens_sb[:].rearrange("p (a b) -> p a b", a=4),
                in_=lens_bcast)

            ident = cp.tile([128, 128], BF16)
            make_identity(nc, ident[:])
            ones100 = cp.tile([1, 100], BF16)
            nc.vector.memset(ones100[:], 1.0)
            ones_col_bf = cp.tile([100, 1], BF16)
            nc.vector.memset(ones_col_bf[:], 1.0)
            h0 = cp.tile([100, BS], BF16)
            nc.vector.memset(h0[:], 0.0)
            st0 = cp.tile([100, BS], F32)
            nc.vector.memset(st0[:], 0.0)
            st0b = cp.tile([100, BS], BF16)
            nc.vector.memset(st0b[:], 0.0)

            # persistent rows
            li_row = bigp.tile([1, SLOTS], BF16)
            tis = [bigp.tile([100, TCH * BS], BF16, tag=f"ti{c}", name=f"ti{c}")
                   for c in range(NCH)]

            xpcs = {}
            rnncs = {}
            zbcs = {}
            lrcs = {}

            # ---------- stage emitters ----------
            gbigs = {}

            def emit_gather_tile(c, j):
                """Gather tile j of chunk c; write xp slices (flat APs)."""
                if j == 0:
                    xpcs[c] = (
                        xpp.tile([100, 2 * TCH * BS], BF16, tag="xprz",
                                 name="xprz"),
                        xpp.tile([100, TCH * BS], BF16, tag="xpn", name="xpn"),
                    )
                xprz, xpn = xpcs[c]
                tile_id = c * GPC + j
                g = gp.tile([128, E], BF16, tag="g", name="g")
                nc.gpsimd.indirect_dma_start(
                    out=g[:], out_offset=None, in_=emb[:],
                    in_offset=bass.IndirectOffsetOnAxis(
                        ap=idx_sb[:, tile_id:tile_id + 1], axis=0),
                )
                rt = rtp.tile([100, 256], BF16, tag="rt", name="rt")
                for ch in range(2):
                    tr = pa.tile([100, 384], BF16, space="PSUM", tag="pa",
                                 name="tr")
                    nc.tensor.transpose(
                        out=tr[:, 0:128],
                        in_=g[:, ch * 100:(ch + 1) * 100],
                        identity=ident[:])
                    if ch == 0:
                        nc.vector.tensor_copy(out=rt[:, 0:128], in_=tr[:, 0:128])
                    else:
                        nc.scalar.copy(out=rt[:, 128:256], in_=tr[:, 0:128])
                for gate in range(3):
                    xg = pa.tile([100, 384], F32, space="PSUM", tag="pa",
                                 name="xg")
                    nc.tensor.matmul(
                        out=xg[:, 0:128], lhsT=wih_sb[:, (0 * 3 + gate) * 100:(0 * 3 + gate) * 100 + 100],
                        rhs=rt[:, 0:128], start=True, stop=False)
                    nc.tensor.matmul(
                        out=xg[:, 0:128], lhsT=wih_sb[:, (1 * 3 + gate) * 100:(1 * 3 + gate) * 100 + 100],
                        rhs=rt[:, 128:256], start=False, stop=True)
                    if gate < 2:
                        dst = xprz[:, gate * 1024 + j * 128:gate * 1024 + (j + 1) * 128]
                        nc.vector.tensor_scalar_add(
                            out=dst, in0=xg[:, 0:128], scalar1=bias_sb[:, gate:gate + 1])
                    else:
                        nc.scalar.activation(
                            out=xpn[:, j * 128:(j + 1) * 128], in_=xg[:, 0:128],
                            func=AF.Identity, bias=bias_sb[:, 2:3])

            def emit_pair(c1, tl, h_prev, st_in, c2):
                """Interleaved scan1 step (c1, tl) + scan2 step (c2, tl).
                c1 None => scan1 done; c2 None => scan2 not started.
                st_in = (st_f32, st_bf16) APs."""
                do1 = c1 is not None
                do2 = c2 is not None
                xprz = xpn = R = P = None
                if do1:
                    xprz, xpn = xpcs[c1]
                    if tl == 0:
                        rnncs[c1] = rnp.tile([100, TCH * BS], BF16, tag="rnnf",
                                             name="rnnf")
                    R = rnncs[c1]
                if do2:
                    st_f, st_b = st_in
                    zbc = zbcs[c2]

                # --- matmuls first (both scans) ---
                if do1:
                    P = p1.tile([100, 96], F32, space="PSUM", tag="p1",
                                name="P")
                    nc.vector.tensor_copy(
                        out=P[:, 0:64],
                        in_=xprz[:].rearrange("p (g r) -> p g r", g=2)[:, :, tl * BS:(tl + 1) * BS])
                    nc.tensor.matmul(out=P[:, 0:32], lhsT=whh_sb[:, 0:100],
                                     rhs=h_prev, start=False, stop=True,
                                     skip_group_check=True)
                    nc.tensor.matmul(out=P[:, 32:64], lhsT=whh_sb[:, 100:200],
                                     rhs=h_prev, start=False, stop=True,
                                     skip_group_check=True)
                    nc.tensor.matmul(out=P[:, 64:96], lhsT=whh_sb[:, 200:300],
                                     rhs=h_prev, start=True, stop=True,
                                     skip_group_check=True)
                if do2:
                    t = c2 * TCH + tl
                    Tg = pg.tile([100, 32], F32, space="PSUM", tag="pg",
                                 name="Tg")
                    Tt = pts.tile([100, 32], F32, space="PSUM", tag="pts",
                                  name="Tt")
                    nc.tensor.matmul(
                        out=Tg[:], lhsT=ones100[:],
                        rhs=li_row[0:1, t * BS:(t + 1) * BS],
                        start=True, stop=False)
                    nc.tensor.matmul(out=Tg[:], lhsT=wlsr_sb[:],
                                     rhs=st_b, start=False, stop=True)
                    nc.vector.tensor_copy(
                        out=Tt[:], in_=tis[c2][:, tl * BS:(tl + 1) * BS])

                # --- sigmoids ---
                if do1:
                    S = s1p.tile([100, 64], BF16, tag="S", name="S")
                    nc.scalar.activation(out=S[:], in_=P[:, 0:64],
                                         func=AF.Sigmoid)
                if do2:
                    sg = s2p.tile([100, 32], BF16, tag="sg", name="sg")
                    nc.scalar.activation(out=sg[:], in_=Tg[:],
                                         func=AF.Sigmoid)

                # --- middles ---
                if do1:
                    # (P_n + b_hh_n) * r, bias folded in via per-partition STT
                    t1 = s1p.tile([100, 32], F32, tag="t1", name="t1")
                    nc.vector.scalar_tensor_tensor(
                        out=t1[:], in0=P[:, 64:96], scalar=bhhn_sb[:],
                        in1=S[:, 0:32], op0=OP.add, op1=OP.mult)
                    t2 = s1p.tile([100, 32], F32, tag="t2", name="t2")
                    nc.vector.tensor_tensor(out=t2[:], in0=t1[:],
                                            in1=xpn[:, tl * BS:(tl + 1) * BS],
                                            op=OP.add)
                    # tail prep off the tanh chain: a = z*h, cm = 1-z
                    a_t = s1p.tile([100, 32], F32, tag="a", name="a")
                    nc.gpsimd.tensor_tensor(out=a_t[:], in0=S[:, 32:64],
                                            in1=h_prev, op=OP.mult)
                    cm_t = s1p.tile([100, 32], F32, tag="cm", name="cm")
                    nc.gpsimd.tensor_scalar(
                        out=cm_t[:], in0=S[:, 32:64], scalar1=-1.0,
                        scalar2=1.0, op0=OP.mult, op1=OP.add)

                if do2:
                    gs = s2p.tile([100, 32], BF16, tag="gs", name="gs")
                    nc.vector.tensor_tensor(out=gs[:], in0=sg[:], in1=st_f,
                                            op=OP.mult)
                    nc.tensor.matmul(out=Tt[:], lhsT=wts_sb[:],
                                     rhs=gs[:], start=False, stop=True,
                                     skip_group_check=True)
                    # hidden-window tail prep on gpsimd
                    m2 = s2p.tile([100, 32], F32, tag="m2", name="m2")
                    nc.gpsimd.tensor_tensor(
                        out=m2[:], in0=zbc[:, tl * BS:(tl + 1) * BS],
                        in1=st_b, op=OP.mult)
                    dd = s2p.tile([100, 32], F32, tag="dd", name="dd")
                    nc.gpsimd.tensor_tensor(out=dd[:], in0=st_f, in1=m2[:],
                                            op=OP.subtract)

                # --- tanhs ---
                if do1:
                    ng = s1p.tile([100, 32], BF16, tag="ng", name="ng")
                    nc.scalar.activation(out=ng[:], in_=t2[:], func=AF.Tanh)
                if do2:
                    ns = s2p.tile([100, 32], BF16, tag="ns", name="ns")
                    nc.scalar.activation(out=ns[:], in_=Tt[:],
                                         func=AF.Tanh)

                # --- tails ---
                h_new = h_prev
                st_out = st_in
                if do1:
                    # h_new = (1-z)*ng + z*h, with both factors precomputed
                    u_t = s1p.tile([100, 32], F32, tag="u", name="u")
                    nc.vector.tensor_tensor(out=u_t[:], in0=ng[:], in1=cm_t[:],
                                            op=OP.mult)
                    h_new = R[:, tl * BS:(tl + 1) * BS]
                    nc.vector.tensor_tensor(out=h_new, in0=u_t[:], in1=a_t[:],
                                            op=OP.add)
                if do2:
                    m1 = s2p.tile([100, 32], F32, tag="m1", name="m1")
                    nc.vector.tensor_tensor(
                        out=m1[:], in0=zbc[:, tl * BS:(tl + 1) * BS],
                        in1=ns[:], op=OP.mult)
                    stf2 = s2p.tile([100, BS], F32, tag="st", name="st")
                    nc.vector.tensor_tensor(out=stf2[:], in0=dd[:], in1=m1[:],
                                            op=OP.add)
                    stb2 = s2p.tile([100, BS], BF16, tag="stb", name="stb")
                    nc.gpsimd.tensor_copy(out=stb2[:], in_=stf2[:])
                    st_out = (stf2[:], stb2[:])
                return h_new, st_out

            def emit_bulk(c):
                """Per-chunk bulk: ti, li, and the cosine-attention z weights.
                Norm chain packs 1024 slots as [8,128] (partition-parallel on
                DVE) via DMA respread; rsqrt is the fast-inverse-sqrt bit trick
                on DVE so the ACT engine never leaves the sigmoid/tanh table
                set (a Sqrt would cost ~5.3us of table reloads per chunk)."""
                R = rnncs[c]
                zbc = zbp.tile([100, TCH * BS], BF16, tag="zbc", name="zbc")
                sq = bkp.tile([100, TCH * BS], BF16, tag="sq", name="sq")
                s12 = bkp.tile([1, 2048], F32, tag="s12", name="s12")
                s1w = bkp.tile([8, 128], F32, tag="s1w", name="s1w")
                s2w = bkp.tile([8, 128], F32, tag="s2w", name="s2w")
                z_row = bkp.tile([1, TCH * BS], BF16, tag="zrow", name="zrow")
                for hh in range(2):
                    sl = slice(hh * 512, (hh + 1) * 512)
                    rows = slice(hh * 4, (hh + 1) * 4)
                    # ti = W_ti@rnn + (b_ti+b_ts)
                    pt = pts.tile([100, 512], F32, space="PSUM", tag="pts",
                                 name="pt")
                    nc.tensor.matmul(out=pt[:], lhsT=wti_sb[:],
                                     rhs=R[:, sl], start=True, stop=True)
                    for q in range(2):
                        nc.vector.tensor_scalar_add(
                            out=tis[c][:, hh * 512 + q * 256:hh * 512 + (q + 1) * 256],
                            in0=pt[:, q * 256:(q + 1) * 256],
                            scalar1=bti_sb[:])
                    # li = rnn@W_lgr[:, :H].T + b_lgr
                    pl = pts.tile([1, 512], F32, space="PSUM", tag="pts",
                                 name="pl")
                    nc.tensor.matmul(out=pl[:], lhsT=wli_sb[:],
                                     rhs=R[:, sl], start=True, stop=True)
                    for q in range(2):
                        nc.scalar.activation(
                            out=li_row[0:1, c * 1024 + hh * 512 + q * 256:
                                       c * 1024 + hh * 512 + (q + 1) * 256],
                            in_=pl[:, q * 256:(q + 1) * 256],
                            func=AF.Identity, bias=blgr_sb[:])
                    # sums of R and R^2 over H, then respread to [4,128]
                    for q in range(2):
                        qsl = slice(hh * 512 + q * 256, hh * 512 + (q + 1) * 256)
                        nc.vector.tensor_tensor(out=sq[:, qsl], in0=R[:, qsl],
                                                in1=R[:, qsl], op=OP.mult)
                    ps1 = pts.tile([1, 512], F32, space="PSUM", tag="pts",
                                  name="ps1")
                    nc.tensor.matmul(out=ps1[:], lhsT=ones_col_bf[:],
                                     rhs=R[:, sl], start=True, stop=True)
                    nc.vector.tensor_copy(out=s12[0:1, sl], in_=ps1[:])
                    nc.sync.dma_start(out=s1w[rows, :], in_=s12[0:1, sl])
                    ps2 = pts.tile([1, 512], F32, space="PSUM", tag="pts",
                                  name="ps2")
                    nc.tensor.matmul(out=ps2[:], lhsT=ones_col_bf[:],
                                     rhs=sq[:, sl], start=True, stop=True)
                    nc.scalar.copy(out=s12[0:1, 1024 + hh * 512:1024 + (hh + 1) * 512],
                                   in_=ps2[:])
                    nc.sync.dma_start(
                        out=s2w[rows, :],
                        in_=s12[0:1, 1024 + hh * 512:1024 + (hh + 1) * 512])
                # rsqrt(s2) on DVE: magic-constant seed + 2 Newton steps
                shi = bkp.tile([8, 128], I32, tag="shi", name="shi")
                nc.vector.tensor_scalar(
                    out=shi[:], in0=s2w[:].bitcast(I32),
                    scalar1=1, scalar2=None, op0=OP.logical_shift_right)
                sei = bkp.tile([8, 128], I32, tag="sei", name="sei")
                nc.vector.tensor_scalar(
                    out=sei[:], in0=shi[:], scalar1=-1,
                    scalar2=0x5F3759DF, op0=OP.mult, op1=OP.add)
                y = sei[:].bitcast(F32)
                for it in range(2):
                    q1 = bkp.tile([8, 128], F32, tag=f"q1{it}", name="q1")
                    nc.vector.tensor_tensor(out=q1[:], in0=y, in1=y,
                                            op=OP.mult)
                    q2 = bkp.tile([8, 128], F32, tag=f"q2{it}", name="q2")
                    nc.vector.tensor_tensor(out=q2[:], in0=q1[:],
                                            in1=s2w[:], op=OP.mult)
                    q3 = bkp.tile([8, 128], F32, tag=f"q3{it}", name="q3")
                    nc.vector.tensor_scalar(
                        out=q3[:], in0=q2[:], scalar1=-0.5,
                        scalar2=1.5, op0=OP.mult, op1=OP.add)
                    yn = bkp.tile([8, 128], F32, tag=f"yn{it}", name="yn")
                    nc.vector.tensor_tensor(out=yn[:], in0=y,
                                            in1=q3[:], op=OP.mult)
                    y = yn[:]
                att = bkp.tile([8, 128], F32, tag="att", name="att")
                nc.vector.tensor_tensor(out=att[:], in0=s1w[:],
                                        in1=y, op=OP.mult)
                z1 = bkp.tile([8, 128], F32, tag="z1", name="z1")
                nc.vector.tensor_scalar(out=z1[:], in0=att[:],
                                        scalar1=0.0, scalar2=1e-3,
                                        op0=OP.max, op1=OP.mult)
                cmp = bkp.tile([8, 128], F32, tag="cmp", name="cmp")
                nc.vector.scalar_tensor_tensor(
                    out=cmp[:], in0=tvals_sb[:],
                    scalar=float(c * TCH), in1=lens_sb[:],
                    op0=OP.add, op1=OP.is_lt)
                zw = bkp.tile([8, 128], BF16, tag="zw", name="zw")
                nc.vector.tensor_tensor(out=zw[:], in0=z1[:],
                                        in1=cmp[:], op=OP.mult)
                nc.sync.dma_start(out=z_row[0:1, :], in_=zw[:])
                # broadcast z to 100 partitions
                for hh in range(2):
                    sl = slice(hh * 512, (hh + 1) * 512)
                    pz = pts.tile([100, 512], F32, space="PSUM", tag="pts",
                                 name="pz")
                    nc.tensor.matmul(out=pz[:], lhsT=ones100[:],
                                     rhs=z_row[0:1, sl], start=True, stop=True)
                    for q in range(2):
                        nc.vector.tensor_copy(
                            out=zbc[:, hh * 512 + q * 256:hh * 512 + (q + 1) * 256],
                            in_=pz[:, q * 256:(q + 1) * 256])
                zbcs[c] = zbc

            # ---------- emit pipeline (interleaved) ----------
            h_prev = h0[:]
            st_cur = (st0[:], st0b[:])
            for j in range(GPC):
                emit_gather_tile(0, j)
            for j in range(GPC):
                emit_gather_tile(1, j)
            for c in range(NCH):
                for tl in range(TCH):
                    if tl % 4 == 0 and c + 2 < NCH:
                        emit_gather_tile(c + 2, tl // 4)
                    h_prev, st_cur = emit_pair(
                        c, tl, h_prev, st_cur, c - 1 if c >= 1 else None)
                emit_bulk(c)
            for tl in range(TCH):
                _, st_cur = emit_pair(None, tl, h_prev, st_cur, NCH - 1)

            # ---------- output ----------
            po = pg.tile([100, 32], F32, space="PSUM", tag="pg", name="po")
            nc.tensor.matmul(out=po[0:NCLS, 0:BS], lhsT=wout_sb[:],
                             rhs=st_cur[0], start=True, stop=True)
            osb = s2p.tile([NCLS, BS], F32, tag="osb", name="osb")
            nc.scalar.activation(out=osb[:], in_=po[0:NCLS, 0:BS],
                                 func=AF.Identity, bias=bout_sb[:])
            nc.sync.dma_start(out=out_d[:], in_=osb[:])

    _split_excess_waits(nc)
    return nc


_NC = None


def _get_nc():
    global _NC
    if _NC is None:
        _NC = build_graph()
    return _NC


def _prep_core_inputs(txt_s, lens_s, shared):
    """Per-core host prep: gather indices + lens."""
    flat = np.ascontiguousarray(txt_s.T).reshape(-1)  # slot k = t*BS + b
    idx_p = np.ascontiguousarray(
        flat.reshape(GT, 128).T).astype(np.int32)      # [p, tile]
    lens_p = lens_s.astype(np.float32).reshape(1, BS)
    m = dict(shared)
    m["idx"] = idx_p
    m["lens"] = lens_p
    return m


def _prep_shared(emb, W_ih, W_hh, b_ih, b_hh, W_lgr, b_lgr, W_ts, b_ts,
                 W_ti, b_ti, W_out, b_out):
    f32 = np.float32
    emb = np.ascontiguousarray(emb, dtype=f32)
    Wg = np.asarray(W_ih, f32).reshape(3, H, E)        # [g, m, e]
    arr = Wg.transpose(2, 0, 1)                        # [e, g, m]
    wih_p = np.ascontiguousarray(
        np.stack([arr[0:100], arr[100:200]], axis=1).reshape(100, 600)
    ).astype(BF)
    Whg = np.asarray(W_hh, f32).reshape(3, H, H)       # [g, m, h]
    whh_p = np.ascontiguousarray(
        Whg.transpose(2, 0, 1).reshape(H, 300)).astype(BF)
    b_ih = np.asarray(b_ih, f32)
    b_hh = np.asarray(b_hh, f32)
    biases_p = np.stack(
        [b_ih[0:H] + b_hh[0:H], b_ih[H:2 * H] + b_hh[H:2 * H], b_ih[2 * H:]],
        axis=1).astype(f32)                            # (100, 3)
    bhhn_p = b_hh[2 * H:].reshape(H, 1).astype(f32)
    wti_p = np.ascontiguousarray(np.asarray(W_ti, f32).T).astype(BF)
    bti_p = (np.asarray(b_ti, f32) + np.asarray(b_ts, f32)).reshape(H, 1)
    W_lgr = np.asarray(W_lgr, f32)
    wls_p = np.ascontiguousarray(np.repeat(W_lgr[0, H:].reshape(H, 1), H, axis=1)).astype(BF)
    wli_p = np.ascontiguousarray(W_lgr[0, :H].reshape(H, 1)).astype(BF)
    blgr_p = np.asarray(b_lgr, f32).reshape(1, 1)
    wts_p = np.ascontiguousarray(np.asarray(W_ts, f32).T).astype(BF)
    wout_p = np.ascontiguousarray(np.asarray(W_out, f32).T).astype(f32)
    bout_p = np.asarray(b_out, f32).reshape(NCLS, 1)
    tv = (np.arange(8)[:, None] * 4 + np.arange(128)[None, :] // 32).astype(f32)
    return {
        "emb": emb, "wih": wih_p, "whh": whh_p, "biases": biases_p,
        "bhhn": bhhn_p, "wti": wti_p, "bti": bti_p, "wls": wls_p,
        "wli": wli_p, "blgr": blgr_p, "wts": wts_p, "wout": wout_p,
        "bout": bout_p, "tvals": tv,
    }


def run(inputs, trace=False):
    txt = np.asarray(inputs["txt"]).astype(np.int32)
    lens = np.asarray(inputs["lens"]).astype(np.int32)
    shared = _prep_shared(
        inputs["emb"], inputs["W_ih"], inputs["W_hh"], inputs["b_ih"],
        inputs["b_hh"], inputs["W_lgr"], inputs["b_lgr"], inputs["W_ts"],
        inputs["b_ts"], inputs["W_ti"], inputs["b_ti"], inputs["W_out"],
        inputs["b_out"])
    in_maps = []
    for core in range(NCORES):
        sl = slice(core * BS, (core + 1) * BS)
        in_maps.append(_prep_core_inputs(txt[sl], lens[sl], shared))
    nc = _get_nc()
    res = run_bass_kernel_spmd(nc, in_maps, core_ids=list(range(NCORES)),
                               trace=trace)
    out = np.empty((B, NCLS), np.float32)
    for core in range(NCORES):
        out[core * BS:(core + 1) * BS] = res.results[core]["out"].T
    return out, res.exec_time_ns


def kernel(**inputs) -> np.ndarray:
    out, _ = run(inputs, trace=False)
    return out



# revision 31
# speedup vs baseline: 1.0489x; 1.0489x over previous
"""Trainium2 Bass kernel for nn_ACWAN (embedding + GRU + cosine-attention +
gated-state recurrence + output projection), data-parallel over batch on 8
NeuronCores.

Self-contained: hardcodes all shapes; host side only reformats weights/indices
and gathers per-core outputs.
"""
import sys

sys.path.insert(0, "/opt/trn_rl_repo")

import numpy as np
import ml_dtypes

import concourse.bass as bass
import concourse.mybir as mybir
from concourse.tile import TileContext
from concourse.bass_utils import run_bass_kernel_spmd
from concourse.masks import make_identity

# ---- problem dims (hardcoded) ----
B, T, E, H, NT, NCLS = 256, 512, 200, 100, 100000, 5
NCORES = 8
BS = B // NCORES            # 32 batch rows per core
TCH = 32                    # timesteps per chunk
NCH = T // TCH              # 16 chunks
SLOTS = BS * T              # 16384 gather slots per core (k = t*BS + b)
GT = SLOTS // 128           # 128 gather tiles (128 rows each)
GPC = GT // NCH             # 8 gather tiles per chunk

F32 = mybir.dt.float32
BF16 = mybir.dt.bfloat16
I32 = mybir.dt.int32
BF = ml_dtypes.bfloat16
AF = mybir.ActivationFunctionType
OP = mybir.AluOpType

MAX_WAITS_DEFAULT = 1


def _split_excess_waits(nc):
    """walrus here accepts very few sem-waits per instruction; hoist extras
    onto NoOps (1 wait each) placed just before, on the same engine."""
    n_fix = 0
    for f in nc.m.functions:
        for bb in f.blocks:
            out = []
            changed = False
            for ins in bb.instructions:
                si = ins.sync_info
                limit = MAX_WAITS_DEFAULT
                if si is not None and si.on_wait and len(si.on_wait) > limit:
                    waits = list(si.on_wait)
                    extra, keep = waits[:-limit], waits[-limit:]
                    for k, w in enumerate(extra):
                        out.append(
                            mybir.InstNoOp(
                                name=f"{ins.name}-wsplit{k}",
                                sync_info=mybir.SyncInfo(on_wait=[w], on_update=[]),
                                bass_nofuse=True,
                                engine=ins.engine,
                            )
                        )
                    ins.sync_info = mybir.SyncInfo(
                        on_wait=keep, on_update=list(si.on_update)
                    )
                    n_fix += 1
                    changed = True
                out.append(ins)
            if changed:
                bb.instructions = out
    return n_fix


def build_graph():
    nc = bass.Bass()

    dp = nc.declare_dram_parameter
    emb = dp("emb", [NT, E], F32, isOutput=False)
    idx = dp("idx", [128, GT], I32, isOutput=False)          # [p, tile]
    wih = dp("wih", [100, 600], BF16, isOutput=False)        # [e, (chunk,gate,m)]
    whh = dp("whh", [100, 300], BF16, isOutput=False)        # [h, (gate,m)]
    biases = dp("biases", [100, 3], F32, isOutput=False)     # r,z fused; n = b_ih_n
    bhhn = dp("bhhn", [100, 1], F32, isOutput=False)         # b_hh n-gate column
    wti = dp("wti", [100, 100], BF16, isOutput=False)
    bti = dp("bti", [100, 1], F32, isOutput=False)           # b_ti + b_ts fused
    wls = dp("wls", [100, 100], F32, isOutput=False)
    wli = dp("wli", [100, 1], BF16, isOutput=False)
    blgr = dp("blgr", [1, 1], F32, isOutput=False)
    wts = dp("wts", [100, 100], F32, isOutput=False)
    wout = dp("wout", [100, NCLS], F32, isOutput=False)
    bout = dp("bout", [NCLS, 1], F32, isOutput=False)
    lens = dp("lens", [1, BS], F32, isOutput=False)
    tvals = dp("tvals", [8, 128], F32, isOutput=False)       # p*4 + f//32
    out_d = dp("out", [NCLS, BS], F32, isOutput=True)

    with TileContext(nc) as tc:
        with tc.tile_pool(name="const", bufs=1) as cp, \
             tc.tile_pool(name="big", bufs=1) as bigp, \
             tc.tile_pool(name="xpring", bufs=4) as xpp, \
             tc.tile_pool(name="rnnring", bufs=3) as rnp, \
             tc.tile_pool(name="zbring", bufs=3) as zbp, \
             tc.tile_pool(name="gring", bufs=6) as gp, \
             tc.tile_pool(name="rtring", bufs=3) as rtp, \
             tc.tile_pool(name="s1sc", bufs=3) as s1p, \
             tc.tile_pool(name="s2sc", bufs=3) as s2p, \
             tc.tile_pool(name="blksc", bufs=1) as bkp, \
             tc.tile_pool(name="pa", bufs=2, space="PSUM") as pa, \
             tc.tile_pool(name="p1", bufs=2, space="PSUM") as p1, \
             tc.tile_pool(name="pg", bufs=2, space="PSUM") as pg, \
             tc.tile_pool(name="pts", bufs=2, space="PSUM") as pts:

            # ---------- load constants ----------
            idx_sb = cp.tile([128, GT], I32)
            nc.sync.dma_start(out=idx_sb[:], in_=idx[:])
            wih_sb = cp.tile([100, 600], BF16)
            nc.sync.dma_start(out=wih_sb[:], in_=wih[:])
            whh_sb = cp.tile([100, 300], BF16)
            nc.sync.dma_start(out=whh_sb[:], in_=whh[:])
            bias_sb = cp.tile([100, 3], F32)
            nc.sync.dma_start(out=bias_sb[:], in_=biases[:])
            bhhn_sb = cp.tile([100, 1], F32)
            nc.sync.dma_start(out=bhhn_sb[:], in_=bhhn[:])
            wti_sb = cp.tile([100, 100], BF16)
            nc.sync.dma_start(out=wti_sb[:], in_=wti[:])
            bti_sb = cp.tile([100, 1], F32)
            nc.sync.dma_start(out=bti_sb[:], in_=bti[:])
            wlsr_sb = cp.tile([100, 100], F32)
            nc.sync.dma_start(out=wlsr_sb[:], in_=wls[:])
            wli_sb = cp.tile([100, 1], BF16)
            nc.sync.dma_start(out=wli_sb[:], in_=wli[:])
            blgr_sb = cp.tile([1, 1], F32)
            nc.sync.dma_start(out=blgr_sb[:], in_=blgr[:])
            wts_sb = cp.tile([100, 100], F32)
            nc.sync.dma_start(out=wts_sb[:], in_=wts[:])
            wout_sb = cp.tile([100, NCLS], F32)
            nc.sync.dma_start(out=wout_sb[:], in_=wout[:])
            bout_sb = cp.tile([NCLS, 1], F32)
            nc.sync.dma_start(out=bout_sb[:], in_=bout[:])
            tvals_sb = cp.tile([8, 128], F32)
            nc.sync.dma_start(out=tvals_sb[:], in_=tvals[:])

            lens_sb = cp.tile([8, 128], F32)
            lens_bcast = bass.AP(
                tensor=lens[:].tensor, offset=0,
                ap=[[0, 8], [0, 4], [1, BS]],
            )
            nc.sync.dma_start(
                out=lens_sb[:].rearrange("p (a b) -> p a b", a=4),
                in_=lens_bcast)

            ident = cp.tile([128, 128], BF16)
            make_identity(nc, ident[:])
            ones100 = cp.tile([1, 100], BF16)
            nc.vector.memset(ones100[:], 1.0)
            ones_col_bf = cp.tile([100, 1], BF16)
            nc.vector.memset(ones_col_bf[:], 1.0)
            h0 = cp.tile([100, BS], BF16)
            nc.vector.memset(h0[:], 0.0)
            st0 = cp.tile([100, BS], F32)
            nc.vector.memset(st0[:], 0.0)

            # persistent rows
            li_row = bigp.tile([1, SLOTS], BF16)
            tis = [bigp.tile([100, TCH * BS], BF16, tag=f"ti{c}", name=f"ti{c}")
                   for c in range(NCH)]

            xpcs = {}
            rnncs = {}
            zbcs = {}
            bulk_y = {}

            # ---------- stage emitters ----------
            gbigs = {}

            gtiles = {}
            rttiles = {}

            def gather_a(c, j):
                """Indirect gather of tile j of chunk c (gpsimd SWDGE)."""
                g = gp.tile([128, E], BF16, tag="g", name="g")
                gtiles[(c, j)] = g
                tile_id = c * GPC + j
                nc.gpsimd.indirect_dma_start(
                    out=g[:], out_offset=None, in_=emb[:],
                    in_offset=bass.IndirectOffsetOnAxis(
                        ap=idx_sb[:, tile_id:tile_id + 1], axis=0),
                )

            def gather_b(c, j):
                """Transpose gathered rows into [E, slots] layout."""
                g = gtiles[(c, j)]
                rt = rtp.tile([100, 256], BF16, tag="rt", name="rt")
                rttiles[(c, j)] = rt
                for ch in range(2):
                    tr = pa.tile([100, 384], BF16, space="PSUM", tag="pa",
                                 name="tr")
                    nc.tensor.transpose(
                        out=tr[:, 0:128],
                        in_=g[:, ch * 100:(ch + 1) * 100],
                        identity=ident[:])
                    if ch == 0:
                        nc.vector.tensor_copy(out=rt[:, 0:128], in_=tr[:, 0:128])
                    else:
                        nc.scalar.copy(out=rt[:, 128:256], in_=tr[:, 0:128])

            def gather_c(c, j, gate):
                """xp matmul for one gate of tile j."""
                xprz, xpn = xpcs[c]
                rt = rttiles[(c, j)]
                xg = pa.tile([100, 384], F32, space="PSUM", tag="pa",
                             name="xg")
                nc.tensor.matmul(
                    out=xg[:, 0:128], lhsT=wih_sb[:, (0 * 3 + gate) * 100:(0 * 3 + gate) * 100 + 100],
                    rhs=rt[:, 0:128], start=True, stop=False)
                nc.tensor.matmul(
                    out=xg[:, 0:128], lhsT=wih_sb[:, (1 * 3 + gate) * 100:(1 * 3 + gate) * 100 + 100],
                    rhs=rt[:, 128:256], start=False, stop=True)
                if gate < 2:
                    dst = xprz[:, gate * 1024 + j * 128:gate * 1024 + (j + 1) * 128]
                    nc.vector.tensor_scalar_add(
                        out=dst, in0=xg[:, 0:128], scalar1=bias_sb[:, gate:gate + 1])
                else:
                    nc.scalar.activation(
                        out=xpn[:, j * 128:(j + 1) * 128], in_=xg[:, 0:128],
                        func=AF.Identity, bias=bias_sb[:, 2:3])

            def gather_pieces(c):
                """Work-queue pieces to prepare chunk c's xp (emitted spread
                across slots: the 1.1us SWDGE op must not head-of-line-block
                the per-step gpsimd tail ops)."""
                xpcs[c] = (
                    xpp.tile([100, 2 * TCH * BS], BF16, tag="xprz",
                             name="xprz"),
                    xpp.tile([100, TCH * BS], BF16, tag="xpn", name="xpn"),
                )
                ps = []
                for j in range(GPC):
                    ps.append(lambda c=c, j=j: gather_a(c, j))
                    ps.append(lambda c=c, j=j: gather_b(c, j))
                    ps.append(lambda c=c, j=j: gather_c(c, j, 0))
                    ps.append(lambda c=c, j=j: (gather_c(c, j, 1),
                                                gather_c(c, j, 2)))
                return ps

            def emit_pair(c1, tl, h_prev, st_in, c2, tl2):
                """Interleaved scan1 step (c1, tl) + scan2 step (c2, tl2).
                c1 None => scan1 done; c2 None => scan2 not started/active.
                st_in = st_f32 AP; scan2 matmuls run in fp32 so no bf16 state
                cast sits on the recurrence chain."""
                do1 = c1 is not None
                do2 = c2 is not None
                xprz = xpn = R = P = None
                if do1:
                    xprz, xpn = xpcs[c1]
                    if tl == 0:
                        rnncs[c1] = rnp.tile([100, TCH * BS], BF16, tag="rnnf",
                                             name="rnnf")
                    R = rnncs[c1]
                if do2:
                    st_f = st_in
                    zbc = zbcs[c2]

                # --- matmuls first (both scans) ---
                if do1:
                    P = p1.tile([100, 96], F32, space="PSUM", tag="p1",
                                name="P")
                    nc.vector.tensor_copy(
                        out=P[:, 0:64],
                        in_=xprz[:].rearrange("p (g r) -> p g r", g=2)[:, :, tl * BS:(tl + 1) * BS])
                    nc.tensor.matmul(out=P[:, 0:32], lhsT=whh_sb[:, 0:100],
                                     rhs=h_prev, start=False, stop=True,
                                     skip_group_check=True)
                    nc.tensor.matmul(out=P[:, 32:64], lhsT=whh_sb[:, 100:200],
                                     rhs=h_prev, start=False, stop=True,
                                     skip_group_check=True)
                    nc.tensor.matmul(out=P[:, 64:96], lhsT=whh_sb[:, 200:300],
                                     rhs=h_prev, start=True, stop=True,
                                     skip_group_check=True)
                if do2:
                    t = c2 * TCH + tl2
                    Tg = pg.tile([100, 32], F32, space="PSUM", tag="pg",
                                 name="Tg")
                    Tt = pg.tile([100, 32], F32, space="PSUM", tag="pg",
                                 name="Tt")
                    nc.tensor.matmul(
                        out=Tg[:], lhsT=ones100[:],
                        rhs=li_row[0:1, t * BS:(t + 1) * BS],
                        start=True, stop=False)
                    nc.tensor.matmul(out=Tg[:], lhsT=wlsr_sb[:],
                                     rhs=st_f, start=False, stop=True,
                                     skip_group_check=True)
                    nc.vector.tensor_copy(
                        out=Tt[:], in_=tis[c2][:, tl2 * BS:(tl2 + 1) * BS])

                # --- sigmoids ---
                if do1:
                    S = s1p.tile([100, 64], BF16, tag="S", name="S")
                    nc.scalar.activation(out=S[:], in_=P[:, 0:64],
                                         func=AF.Sigmoid)
                if do2:
                    sg = s2p.tile([100, 32], F32, tag="sg", name="sg")
                    nc.scalar.activation(out=sg[:], in_=Tg[:],
                                         func=AF.Sigmoid)

                # --- middles ---
                if do1:
                    # (P_n + b_hh_n) * r, bias folded in via per-partition STT
                    t1 = s1p.tile([100, 32], F32, tag="t1", name="t1")
                    nc.vector.scalar_tensor_tensor(
                        out=t1[:], in0=P[:, 64:96], scalar=bhhn_sb[:],
                        in1=S[:, 0:32], op0=OP.add, op1=OP.mult)
                    t2 = s1p.tile([100, 32], F32, tag="t2", name="t2")
                    nc.vector.tensor_tensor(out=t2[:], in0=t1[:],
                                            in1=xpn[:, tl * BS:(tl + 1) * BS],
                                            op=OP.add)
                    # tail prep off the tanh chain: a = z*h, cm = 1-z
                    a_t = s1p.tile([100, 32], F32, tag="a", name="a")
                    nc.gpsimd.tensor_tensor(out=a_t[:], in0=S[:, 32:64],
                                            in1=h_prev, op=OP.mult)
                    cm_t = s1p.tile([100, 32], F32, tag="cm", name="cm")
                    nc.gpsimd.tensor_scalar(
                        out=cm_t[:], in0=S[:, 32:64], scalar1=-1.0,
                        scalar2=1.0, op0=OP.mult, op1=OP.add)

                if do2:
                    gs = s2p.tile([100, 32], F32, tag="gs", name="gs")
                    nc.vector.tensor_tensor(out=gs[:], in0=sg[:], in1=st_f,
                                            op=OP.mult)
                    nc.tensor.matmul(out=Tt[:], lhsT=wts_sb[:],
                                     rhs=gs[:], start=False, stop=True,
                                     skip_group_check=True)
                    # hidden-window tail prep on gpsimd
                    m2 = s2p.tile([100, 32], F32, tag="m2", name="m2")
                    nc.gpsimd.tensor_tensor(
                        out=m2[:], in0=zbc[:, tl2 * BS:(tl2 + 1) * BS],
                        in1=st_f, op=OP.mult)
                    dd = s2p.tile([100, 32], F32, tag="dd", name="dd")
                    nc.gpsimd.tensor_tensor(out=dd[:], in0=st_f, in1=m2[:],
                                            op=OP.subtract)

                # --- tanhs ---
                if do1:
                    ng = s1p.tile([100, 32], BF16, tag="ng", name="ng")
                    nc.scalar.activation(out=ng[:], in_=t2[:], func=AF.Tanh)
                if do2:
                    ns = s2p.tile([100, 32], BF16, tag="ns", name="ns")
                    nc.scalar.activation(out=ns[:], in_=Tt[:],
                                         func=AF.Tanh)

                # --- tails ---
                h_new = h_prev
                st_out = st_in
                if do1:
                    # h_new = (1-z)*ng + z*h, with both factors precomputed
                    u_t = s1p.tile([100, 32], F32, tag="u", name="u")
                    nc.vector.tensor_tensor(out=u_t[:], in0=ng[:], in1=cm_t[:],
                                            op=OP.mult)
                    h_new = R[:, tl * BS:(tl + 1) * BS]
                    nc.vector.tensor_tensor(out=h_new, in0=u_t[:], in1=a_t[:],
                                            op=OP.add)
                if do2:
                    m1 = s2p.tile([100, 32], F32, tag="m1", name="m1")
                    nc.vector.tensor_tensor(
                        out=m1[:], in0=zbc[:, tl2 * BS:(tl2 + 1) * BS],
                        in1=ns[:], op=OP.mult)
                    stf2 = s2p.tile([100, BS], F32, tag="st", name="st")
                    nc.vector.tensor_tensor(out=stf2[:], in0=dd[:], in1=m1[:],
                                            op=OP.add)
                    st_out = stf2[:]
                return h_new, st_out

            def emit_bulk_pieces(c):
                """Work-queue pieces for per-chunk bulk: ti, li, and the
                cosine-attention z weights. Norm chain packs 1024 slots as
                [8,128] (partition-parallel on DVE) via DMA respread; rsqrt is
                the fast-inverse-sqrt bit trick on DVE so the ACT engine never
                leaves the sigmoid/tanh table set (a Sqrt costs ~5.3us of
                table reloads per chunk). z pieces come first so the next
                chunk's scan2 unblocks earliest."""
                R = rnncs[c]
                zbc = zbp.tile([100, TCH * BS], BF16, tag="zbc", name="zbc")
                zbcs[c] = zbc
                sq = bkp.tile([100, TCH * BS], BF16, tag="sq", name="sq")
                s12 = bkp.tile([1, 2048], F32, tag="s12", name="s12")
                s1w = bkp.tile([8, 128], F32, tag="s1w", name="s1w")
                s2w = bkp.tile([8, 128], F32, tag="s2w", name="s2w")
                z_row = bkp.tile([1, TCH * BS], BF16, tag="zrow", name="zrow")
                ps = []

                def sums(hh):
                    sl = slice(hh * 512, (hh + 1) * 512)
                    rows = slice(hh * 4, (hh + 1) * 4)
                    for q in range(2):
                        qsl = slice(hh * 512 + q * 256, hh * 512 + (q + 1) * 256)
                        nc.vector.tensor_tensor(out=sq[:, qsl], in0=R[:, qsl],
                                                in1=R[:, qsl], op=OP.mult)
                    ps1 = pts.tile([1, 512], F32, space="PSUM", tag="pts",
                                  name="ps1")
                    nc.tensor.matmul(out=ps1[:], lhsT=ones_col_bf[:],
                                     rhs=R[:, sl], start=True, stop=True)
                    nc.vector.tensor_copy(out=s12[0:1, sl], in_=ps1[:])
                    nc.sync.dma_start(out=s1w[rows, :], in_=s12[0:1, sl])

                def sums2(hh):
                    sl = slice(hh * 512, (hh + 1) * 512)
                    rows = slice(hh * 4, (hh + 1) * 4)
                    ps2 = pts.tile([1, 512], F32, space="PSUM", tag="pts",
                                  name="ps2")
                    nc.tensor.matmul(out=ps2[:], lhsT=ones_col_bf[:],
                                     rhs=sq[:, sl], start=True, stop=True)
                    nc.scalar.copy(out=s12[0:1, 1024 + hh * 512:1024 + (hh + 1) * 512],
                                   in_=ps2[:])
                    nc.sync.dma_start(
                        out=s2w[rows, :],
                        in_=s12[0:1, 1024 + hh * 512:1024 + (hh + 1) * 512])

                ps.append(lambda: sums(0))
                ps.append(lambda: sums2(0))
                ps.append(lambda: sums(1))
                ps.append(lambda: sums2(1))

                def rsq_seed():
                    shi = bkp.tile([8, 128], I32, tag="shi", name="shi")
                    nc.vector.tensor_scalar(
                        out=shi[:], in0=s2w[:].bitcast(I32),
                        scalar1=1, scalar2=None, op0=OP.logical_shift_right)
                    sei = bkp.tile([8, 128], I32, tag="sei", name="sei")
                    nc.vector.tensor_scalar(
                        out=sei[:], in0=shi[:], scalar1=-1,
                        scalar2=0x5F3759DF, op0=OP.mult, op1=OP.add)
                    bulk_y[c] = sei[:].bitcast(F32)

                def rsq_nr(it):
                    y = bulk_y[c]
                    q1 = bkp.tile([8, 128], F32, tag=f"q1{it}", name="q1")
                    nc.vector.tensor_tensor(out=q1[:], in0=y, in1=y,
                                            op=OP.mult)
                    q2 = bkp.tile([8, 128], F32, tag=f"q2{it}", name="q2")
                    nc.vector.tensor_tensor(out=q2[:], in0=q1[:],
                                            in1=s2w[:], op=OP.mult)
                    q3 = bkp.tile([8, 128], F32, tag=f"q3{it}", name="q3")
                    nc.vector.tensor_scalar(
                        out=q3[:], in0=q2[:], scalar1=-0.5,
                        scalar2=1.5, op0=OP.mult, op1=OP.add)
                    yn = bkp.tile([8, 128], F32, tag=f"yn{it}", name="yn")
                    nc.vector.tensor_tensor(out=yn[:], in0=y,
                                            in1=q3[:], op=OP.mult)
                    bulk_y[c] = yn[:]

                def zfinish():
                    y = bulk_y[c]
                    att = bkp.tile([8, 128], F32, tag="att", name="att")
                    nc.vector.tensor_tensor(out=att[:], in0=s1w[:],
                                            in1=y, op=OP.mult)
                    z1 = bkp.tile([8, 128], F32, tag="z1", name="z1")
                    nc.vector.tensor_scalar(out=z1[:], in0=att[:],
                                            scalar1=0.0, scalar2=1e-3,
                                            op0=OP.max, op1=OP.mult)
                    cmp = bkp.tile([8, 128], F32, tag="cmp", name="cmp")
                    nc.vector.scalar_tensor_tensor(
                        out=cmp[:], in0=tvals_sb[:],
                        scalar=float(c * TCH), in1=lens_sb[:],
                        op0=OP.add, op1=OP.is_lt)
                    zw = bkp.tile([8, 128], BF16, tag="zw", name="zw")
                    nc.vector.tensor_tensor(out=zw[:], in0=z1[:],
                                            in1=cmp[:], op=OP.mult)
                    nc.sync.dma_start(out=z_row[0:1, :], in_=zw[:])

                ps.append(rsq_seed)
                ps.append(lambda: rsq_nr(0))
                ps.append(lambda: rsq_nr(1))
                ps.append(zfinish)

                def zbcast(hh):
                    sl = slice(hh * 512, (hh + 1) * 512)
                    pz = pts.tile([100, 512], F32, space="PSUM", tag="pts",
                                 name="pz")
                    nc.tensor.matmul(out=pz[:], lhsT=ones100[:],
                                     rhs=z_row[0:1, sl], start=True, stop=True)
                    for q in range(2):
                        nc.vector.tensor_copy(
                            out=zbc[:, hh * 512 + q * 256:hh * 512 + (q + 1) * 256],
                            in_=pz[:, q * 256:(q + 1) * 256])

                def ti_piece(hh):
                    pt = pts.tile([100, 512], F32, space="PSUM", tag="pts",
                                 name="pt")
                    nc.tensor.matmul(out=pt[:], lhsT=wti_sb[:],
                                     rhs=R[:, hh * 512:(hh + 1) * 512],
                                     start=True, stop=True)
                    for q in range(2):
                        nc.vector.tensor_scalar_add(
                            out=tis[c][:, hh * 512 + q * 256:hh * 512 + (q + 1) * 256],
                            in0=pt[:, q * 256:(q + 1) * 256],
                            scalar1=bti_sb[:])

                def li_piece(hh):
                    pl = pts.tile([1, 512], F32, space="PSUM", tag="pts",
                                 name="pl")
                    nc.tensor.matmul(out=pl[:], lhsT=wli_sb[:],
                                     rhs=R[:, hh * 512:(hh + 1) * 512],
                                     start=True, stop=True)
                    for q in range(2):
                        nc.scalar.activation(
                            out=li_row[0:1, c * 1024 + hh * 512 + q * 256:
                                       c * 1024 + hh * 512 + (q + 1) * 256],
                            in_=pl[:, q * 256:(q + 1) * 256],
                            func=AF.Identity, bias=blgr_sb[:])

                ps.append(lambda: zbcast(0))
                ps.append(lambda: ti_piece(0))
                ps.append(lambda: li_piece(0))
                ps.append(lambda: zbcast(1))
                ps.append(lambda: ti_piece(1))
                ps.append(lambda: li_piece(1))
                return ps

            # ---------- emit pipeline (interleaved) ----------
            # scan1 runs at slot S; scan2 trails by LAG slots (one chunk plus
            # 12 slots of headroom so the spread bulk pieces finish in time).
            # Gather/xp/bulk work drains from a FIFO a few pieces per slot so
            # no engine queue ever sees a long burst ahead of chain ops.
            h_prev = h0[:]
            st_cur = st0[:]
            for piece in gather_pieces(0):
                piece()
            workq = []
            workq.extend(gather_pieces(1))
            LAG = TCH + 12
            TOT = NCH * TCH
            for S in range(TOT + LAG):
                if S < TOT:
                    c1, tl = divmod(S, TCH)
                    if tl == 0 and c1 + 2 < NCH:
                        workq.extend(gather_pieces(c1 + 2))
                else:
                    c1, tl = None, None
                u = S - LAG
                if u >= 0:
                    c2, tl2 = divmod(u, TCH)
                else:
                    c2, tl2 = None, None
                h_prev, st_cur = emit_pair(c1, tl, h_prev, st_cur, c2, tl2)
                if c1 is not None and tl == TCH - 1:
                    workq.extend(emit_bulk_pieces(c1))
                npop = 3 if (c1 is None or len(workq) > 20) else 2
                for _ in range(min(npop, len(workq))):
                    workq.pop(0)()

            # ---------- output ----------
            po = pg.tile([100, 32], F32, space="PSUM", tag="pg", name="po")
            nc.tensor.matmul(out=po[0:NCLS, 0:BS], lhsT=wout_sb[:],
                             rhs=st_cur, start=True, stop=True)
            osb = s2p.tile([NCLS, BS], F32, tag="osb", name="osb")
            nc.scalar.activation(out=osb[:], in_=po[0:NCLS, 0:BS],
                                 func=AF.Identity, bias=bout_sb[:])
            nc.sync.dma_start(out=out_d[:], in_=osb[:])

    _split_excess_waits(nc)
    return nc


_NC = None


def _get_nc():
    global _NC
    if _NC is None:
        _NC = build_graph()
    return _NC


def _prep_core_inputs(txt_s, lens_s, shared):
    """Per-core host prep: gather indices + lens."""
    flat = np.ascontiguousarray(txt_s.T).reshape(-1)  # slot k = t*BS + b
    idx_p = np.ascontiguousarray(
        flat.reshape(GT, 128).T).astype(np.int32)      # [p, tile]
    lens_p = lens_s.astype(np.float32).reshape(1, BS)
    m = dict(shared)
    m["idx"] = idx_p
    m["lens"] = lens_p
    return m


def _prep_shared(emb, W_ih, W_hh, b_ih, b_hh, W_lgr, b_lgr, W_ts, b_ts,
                 W_ti, b_ti, W_out, b_out):
    f32 = np.float32
    emb = np.ascontiguousarray(emb, dtype=f32)
    Wg = np.asarray(W_ih, f32).reshape(3, H, E)        # [g, m, e]
    arr = Wg.transpose(2, 0, 1)                        # [e, g, m]
    wih_p = np.ascontiguousarray(
        np.stack([arr[0:100], arr[100:200]], axis=1).reshape(100, 600)
    ).astype(BF)
    Whg = np.asarray(W_hh, f32).reshape(3, H, H)       # [g, m, h]
    whh_p = np.ascontiguousarray(
        Whg.transpose(2, 0, 1).reshape(H, 300)).astype(BF)
    b_ih = np.asarray(b_ih, f32)
    b_hh = np.asarray(b_hh, f32)
    biases_p = np.stack(
        [b_ih[0:H] + b_hh[0:H], b_ih[H:2 * H] + b_hh[H:2 * H], b_ih[2 * H:]],
        axis=1).astype(f32)                            # (100, 3)
    bhhn_p = b_hh[2 * H:].reshape(H, 1).astype(f32)
    wti_p = np.ascontiguousarray(np.asarray(W_ti, f32).T).astype(BF)
    bti_p = (np.asarray(b_ti, f32) + np.asarray(b_ts, f32)).reshape(H, 1)
    W_lgr = np.asarray(W_lgr, f32)
    wls_p = np.ascontiguousarray(np.repeat(W_lgr[0, H:].reshape(H, 1), H, axis=1)).astype(f32)
    wli_p = np.ascontiguousarray(W_lgr[0, :H].reshape(H, 1)).astype(BF)
    blgr_p = np.asarray(b_lgr, f32).reshape(1, 1)
    wts_p = np.ascontiguousarray(np.asarray(W_ts, f32).T).astype(f32)
    wout_p = np.ascontiguousarray(np.asarray(W_out, f32).T).astype(f32)
    bout_p = np.asarray(b_out, f32).reshape(NCLS, 1)
    tv = (np.arange(8)[:, None] * 4 + np.arange(128)[None, :] // 32).astype(f32)
    return {
        "emb": emb, "wih": wih_p, "whh": whh_p, "biases": biases_p,
        "bhhn": bhhn_p, "wti": wti_p, "bti": bti_p, "wls": wls_p,
        "wli": wli_p, "blgr": blgr_p, "wts": wts_p, "wout": wout_p,
        "bout": bout_p, "tvals": tv,
    }


def run(inputs, trace=False):
    txt = np.asarray(inputs["txt"]).astype(np.int32)
    lens = np.asarray(inputs["lens"]).astype(np.int32)
    shared = _prep_shared(
        inputs["emb"], inputs["W_ih"], inputs["W_hh"], inputs["b_ih"],
        inputs["b_hh"], inputs["W_lgr"], inputs["b_lgr"], inputs["W_ts"],
        inputs["b_ts"], inputs["W_ti"], inputs["b_ti"], inputs["W_out"],
        inputs["b_out"])
    in_maps = []
    for core in range(NCORES):
        sl = slice(core * BS, (core + 1) * BS)
        in_maps.append(_prep_core_inputs(txt[sl], lens[sl], shared))
    nc = _get_nc()
    res = run_bass_kernel_spmd(nc, in_maps, core_ids=list(range(NCORES)),
                               trace=trace)
    out = np.empty((B, NCLS), np.float32)
    for core in range(NCORES):
        out[core * BS:(core + 1) * BS] = res.results[core]["out"].T
    return out, res.exec_time_ns


def kernel(**inputs) -> np.ndarray:
    out, _ = run(inputs, trace=False)
    return out



# revision 33
# speedup vs baseline: 1.1783x; 1.1234x over previous
"""Trainium2 Bass kernel for nn_ACWAN (embedding + GRU + cosine-attention +
gated-state recurrence + output projection), data-parallel over batch on 8
NeuronCores.

Self-contained: hardcodes all shapes; host side only reformats weights/indices
and gathers per-core outputs.
"""
import sys

sys.path.insert(0, "/opt/trn_rl_repo")

import numpy as np
import ml_dtypes

import concourse.bass as bass
import concourse.mybir as mybir
from concourse.tile import TileContext
from concourse.bass_utils import run_bass_kernel_spmd
from concourse.masks import make_identity

# ---- problem dims (hardcoded) ----
B, T, E, H, NT, NCLS = 256, 512, 200, 100, 100000, 5
NCORES = 8
BS = B // NCORES            # 32 batch rows per core
TCH = 32                    # timesteps per chunk
NCH = T // TCH              # 16 chunks
SLOTS = BS * T              # 16384 gather slots per core (k = t*BS + b)
GT = SLOTS // 128           # 128 gather tiles (128 rows each)
GPC = GT // NCH             # 8 gather tiles per chunk

F32 = mybir.dt.float32
BF16 = mybir.dt.bfloat16
I32 = mybir.dt.int32
BF = ml_dtypes.bfloat16
AF = mybir.ActivationFunctionType
OP = mybir.AluOpType

MAX_WAITS_DEFAULT = 1


def _split_excess_waits(nc):
    """walrus here accepts very few sem-waits per instruction; hoist extras
    onto NoOps (1 wait each) placed just before, on the same engine."""
    n_fix = 0
    for f in nc.m.functions:
        for bb in f.blocks:
            out = []
            changed = False
            for ins in bb.instructions:
                si = ins.sync_info
                limit = MAX_WAITS_DEFAULT
                if si is not None and si.on_wait and len(si.on_wait) > limit:
                    waits = list(si.on_wait)
                    extra, keep = waits[:-limit], waits[-limit:]
                    for k, w in enumerate(extra):
                        out.append(
                            mybir.InstNoOp(
                                name=f"{ins.name}-wsplit{k}",
                                sync_info=mybir.SyncInfo(on_wait=[w], on_update=[]),
                                bass_nofuse=True,
                                engine=ins.engine,
                            )
                        )
                    ins.sync_info = mybir.SyncInfo(
                        on_wait=keep, on_update=list(si.on_update)
                    )
                    n_fix += 1
                    changed = True
                out.append(ins)
            if changed:
                bb.instructions = out
    return n_fix


def build_graph():
    nc = bass.Bass()

    dp = nc.declare_dram_parameter
    emb = dp("emb", [NT, E], F32, isOutput=False)
    idx = dp("idx", [128, GT], I32, isOutput=False)          # [p, tile]
    wih = dp("wih", [100, 600], BF16, isOutput=False)        # [e, (chunk,gate,m)]
    whh = dp("whh", [100, 300], BF16, isOutput=False)        # [h, (gate,m)]
    biases = dp("biases", [100, 3], F32, isOutput=False)     # r,z fused; n = b_ih_n
    bhhn = dp("bhhn", [100, 1], F32, isOutput=False)         # b_hh n-gate column
    wti = dp("wti", [100, 100], BF16, isOutput=False)
    bti = dp("bti", [100, 1], F32, isOutput=False)           # b_ti + b_ts fused
    wls = dp("wls", [100, 100], BF16, isOutput=False)
    wli = dp("wli", [100, 1], BF16, isOutput=False)
    blgr = dp("blgr", [1, 1], F32, isOutput=False)
    wts = dp("wts", [100, 100], BF16, isOutput=False)
    wout = dp("wout", [100, NCLS], F32, isOutput=False)
    bout = dp("bout", [NCLS, 1], F32, isOutput=False)
    lens = dp("lens", [1, BS], F32, isOutput=False)
    tvals = dp("tvals", [8, 128], F32, isOutput=False)       # p*4 + f//32
    out_d = dp("out", [NCLS, BS], F32, isOutput=True)

    with TileContext(nc) as tc:
        with tc.tile_pool(name="const", bufs=1) as cp, \
             tc.tile_pool(name="big", bufs=1) as bigp, \
             tc.tile_pool(name="xpring", bufs=4) as xpp, \
             tc.tile_pool(name="rnnring", bufs=3) as rnp, \
             tc.tile_pool(name="zbring", bufs=3) as zbp, \
             tc.tile_pool(name="gring", bufs=6) as gp, \
             tc.tile_pool(name="rtring", bufs=3) as rtp, \
             tc.tile_pool(name="s1sc", bufs=3) as s1p, \
             tc.tile_pool(name="s2sc", bufs=3) as s2p, \
             tc.tile_pool(name="blksc", bufs=1) as bkp, \
             tc.tile_pool(name="pa", bufs=2, space="PSUM") as pa, \
             tc.tile_pool(name="p1", bufs=2, space="PSUM") as p1, \
             tc.tile_pool(name="pg", bufs=2, space="PSUM") as pg, \
             tc.tile_pool(name="pts", bufs=2, space="PSUM") as pts:

            # ---------- load constants ----------
            idx_sb = cp.tile([128, GT], I32)
            nc.sync.dma_start(out=idx_sb[:], in_=idx[:])
            wih_sb = cp.tile([100, 600], BF16)
            nc.sync.dma_start(out=wih_sb[:], in_=wih[:])
            whh_sb = cp.tile([100, 300], BF16)
            nc.sync.dma_start(out=whh_sb[:], in_=whh[:])
            bias_sb = cp.tile([100, 3], F32)
            nc.sync.dma_start(out=bias_sb[:], in_=biases[:])
            bhhn_sb = cp.tile([100, 1], F32)
            nc.sync.dma_start(out=bhhn_sb[:], in_=bhhn[:])
            wti_sb = cp.tile([100, 100], BF16)
            nc.sync.dma_start(out=wti_sb[:], in_=wti[:])
            bti_sb = cp.tile([100, 1], F32)
            nc.sync.dma_start(out=bti_sb[:], in_=bti[:])
            wlsr_sb = cp.tile([100, 100], BF16)
            nc.sync.dma_start(out=wlsr_sb[:], in_=wls[:])
            wli_sb = cp.tile([100, 1], BF16)
            nc.sync.dma_start(out=wli_sb[:], in_=wli[:])
            blgr_sb = cp.tile([1, 1], F32)
            nc.sync.dma_start(out=blgr_sb[:], in_=blgr[:])
            wts_sb = cp.tile([100, 100], BF16)
            nc.sync.dma_start(out=wts_sb[:], in_=wts[:])
            wout_sb = cp.tile([100, NCLS], F32)
            nc.sync.dma_start(out=wout_sb[:], in_=wout[:])
            bout_sb = cp.tile([NCLS, 1], F32)
            nc.sync.dma_start(out=bout_sb[:], in_=bout[:])
            tvals_sb = cp.tile([8, 128], F32)
            nc.sync.dma_start(out=tvals_sb[:], in_=tvals[:])

            lens_sb = cp.tile([8, 128], F32)
            lens_bcast = bass.AP(
                tensor=lens[:].tensor, offset=0,
                ap=[[0, 8], [0, 4], [1, BS]],
            )
            nc.sync.dma_start(
                out=lens_sb[:].rearrange("p (a b) -> p a b", a=4),
                in_=lens_bcast)

            ident = cp.tile([128, 128], BF16)
            make_identity(nc, ident[:])
            ones100 = cp.tile([1, 100], BF16)
            nc.vector.memset(ones100[:], 1.0)
            ones_col_bf = cp.tile([100, 1], BF16)
            nc.vector.memset(ones_col_bf[:], 1.0)
            h0 = cp.tile([100, BS], BF16)
            nc.vector.memset(h0[:], 0.0)
            st0 = cp.tile([100, BS], F32)
            nc.vector.memset(st0[:], 0.0)
            st0b = cp.tile([100, BS], BF16)
            nc.vector.memset(st0b[:], 0.0)

            # persistent rows
            li_row = bigp.tile([1, SLOTS], BF16)
            tis = [bigp.tile([100, TCH * BS], BF16, tag=f"ti{c}", name=f"ti{c}")
                   for c in range(NCH)]

            xpcs = {}
            rnncs = {}
            zbcs = {}
            bulk_y = {}

            # ---------- stage emitters ----------
            gbigs = {}

            gtiles = {}
            rttiles = {}

            def gather_a(c, j):
                """Indirect gather of tile j of chunk c (gpsimd SWDGE)."""
                g = gp.tile([128, E], BF16, tag="g", name="g")
                gtiles[(c, j)] = g
                tile_id = c * GPC + j
                nc.gpsimd.indirect_dma_start(
                    out=g[:], out_offset=None, in_=emb[:],
                    in_offset=bass.IndirectOffsetOnAxis(
                        ap=idx_sb[:, tile_id:tile_id + 1], axis=0),
                )

            def gather_b(c, j):
                """Transpose gathered rows into [E, slots] layout."""
                g = gtiles[(c, j)]
                rt = rtp.tile([100, 256], BF16, tag="rt", name="rt")
                rttiles[(c, j)] = rt
                for ch in range(2):
                    tr = pa.tile([100, 384], BF16, space="PSUM", tag="pa",
                                 name="tr")
                    nc.tensor.transpose(
                        out=tr[:, 0:128],
                        in_=g[:, ch * 100:(ch + 1) * 100],
                        identity=ident[:])
                    if ch == 0:
                        nc.vector.tensor_copy(out=rt[:, 0:128], in_=tr[:, 0:128])
                    else:
                        nc.scalar.copy(out=rt[:, 128:256], in_=tr[:, 0:128])

            def gather_c(c, j, gate):
                """xp matmul for one gate of tile j."""
                xprz, xpn = xpcs[c]
                rt = rttiles[(c, j)]
                xg = pa.tile([100, 384], F32, space="PSUM", tag="pa",
                             name="xg")
                nc.tensor.matmul(
                    out=xg[:, 0:128], lhsT=wih_sb[:, (0 * 3 + gate) * 100:(0 * 3 + gate) * 100 + 100],
                    rhs=rt[:, 0:128], start=True, stop=False)
                nc.tensor.matmul(
                    out=xg[:, 0:128], lhsT=wih_sb[:, (1 * 3 + gate) * 100:(1 * 3 + gate) * 100 + 100],
                    rhs=rt[:, 128:256], start=False, stop=True)
                if gate < 2:
                    dst = xprz[:, gate * 1024 + j * 128:gate * 1024 + (j + 1) * 128]
                    nc.vector.tensor_scalar_add(
                        out=dst, in0=xg[:, 0:128], scalar1=bias_sb[:, gate:gate + 1])
                else:
                    nc.scalar.activation(
                        out=xpn[:, j * 128:(j + 1) * 128], in_=xg[:, 0:128],
                        func=AF.Identity, bias=bias_sb[:, 2:3])

            def gather_pieces(c):
                """Work-queue pieces to prepare chunk c's xp (emitted spread
                across slots: the 1.1us SWDGE op must not head-of-line-block
                the per-step gpsimd tail ops)."""
                xpcs[c] = (
                    xpp.tile([100, 2 * TCH * BS], BF16, tag="xprz",
                             name="xprz"),
                    xpp.tile([100, TCH * BS], BF16, tag="xpn", name="xpn"),
                )
                ps = []
                for j in range(GPC):
                    ps.append(lambda c=c, j=j: gather_a(c, j))
                    ps.append(lambda c=c, j=j: gather_b(c, j))
                    ps.append(lambda c=c, j=j: gather_c(c, j, 0))
                    ps.append(lambda c=c, j=j: (gather_c(c, j, 1),
                                                gather_c(c, j, 2)))
                return ps

            def emit_pair(c1, tl, h_prev, st_in, c2, tl2):
                """Interleaved scan1 step (c1, tl) + scan2 step (c2, tl2).
                c1 None => scan1 done; c2 None => scan2 not started/active.
                st_in = (st_f32, st_bf16) APs; the bf16 cast for the next
                gate matmul runs on the pipelined DVE."""
                do1 = c1 is not None
                do2 = c2 is not None
                xprz = xpn = R = P = None
                if do1:
                    xprz, xpn = xpcs[c1]
                    if tl == 0:
                        rnncs[c1] = rnp.tile([100, TCH * BS], BF16, tag="rnnf",
                                             name="rnnf")
                    R = rnncs[c1]
                if do2:
                    st_f, st_b = st_in
                    zbc = zbcs[c2]

                # --- matmuls first (both scans) ---
                if do1:
                    P = p1.tile([100, 96], F32, space="PSUM", tag="p1",
                                name="P")
                    nc.vector.tensor_copy(
                        out=P[:, 0:64],
                        in_=xprz[:].rearrange("p (g r) -> p g r", g=2)[:, :, tl * BS:(tl + 1) * BS])
                    nc.tensor.matmul(out=P[:, 0:32], lhsT=whh_sb[:, 0:100],
                                     rhs=h_prev, start=False, stop=True,
                                     skip_group_check=True)
                    nc.tensor.matmul(out=P[:, 32:64], lhsT=whh_sb[:, 100:200],
                                     rhs=h_prev, start=False, stop=True,
                                     skip_group_check=True)
                    nc.tensor.matmul(out=P[:, 64:96], lhsT=whh_sb[:, 200:300],
                                     rhs=h_prev, start=True, stop=True,
                                     skip_group_check=True)
                if do2:
                    t = c2 * TCH + tl2
                    Tg = pg.tile([100, 32], F32, space="PSUM", tag="pg",
                                 name="Tg")
                    Tt = pg.tile([100, 32], F32, space="PSUM", tag="pg",
                                 name="Tt")
                    nc.tensor.matmul(
                        out=Tg[:], lhsT=ones100[:],
                        rhs=li_row[0:1, t * BS:(t + 1) * BS],
                        start=True, stop=False)
                    nc.tensor.matmul(out=Tg[:], lhsT=wlsr_sb[:],
                                     rhs=st_b, start=False, stop=True,
                                     skip_group_check=True)
                    nc.vector.tensor_copy(
                        out=Tt[:], in_=tis[c2][:, tl2 * BS:(tl2 + 1) * BS])

                # --- sigmoids ---
                if do1:
                    S = s1p.tile([100, 64], BF16, tag="S", name="S")
                    nc.scalar.activation(out=S[:], in_=P[:, 0:64],
                                         func=AF.Sigmoid)
                if do2:
                    sg = s2p.tile([100, 32], BF16, tag="sg", name="sg")
                    nc.scalar.activation(out=sg[:], in_=Tg[:],
                                         func=AF.Sigmoid)

                # --- middles ---
                if do1:
                    # (P_n + b_hh_n) * r, bias folded in via per-partition STT
                    t1 = s1p.tile([100, 32], F32, tag="t1", name="t1")
                    nc.vector.scalar_tensor_tensor(
                        out=t1[:], in0=P[:, 64:96], scalar=bhhn_sb[:],
                        in1=S[:, 0:32], op0=OP.add, op1=OP.mult)
                    t2 = s1p.tile([100, 32], F32, tag="t2", name="t2")
                    nc.vector.tensor_tensor(out=t2[:], in0=t1[:],
                                            in1=xpn[:, tl * BS:(tl + 1) * BS],
                                            op=OP.add)
                    # tail prep off the tanh chain: a = z*h, cm = 1-z
                    a_t = s1p.tile([100, 32], F32, tag="a", name="a")
                    nc.gpsimd.tensor_tensor(out=a_t[:], in0=S[:, 32:64],
                                            in1=h_prev, op=OP.mult)
                    cm_t = s1p.tile([100, 32], F32, tag="cm", name="cm")
                    nc.gpsimd.tensor_scalar(
                        out=cm_t[:], in0=S[:, 32:64], scalar1=-1.0,
                        scalar2=1.0, op0=OP.mult, op1=OP.add)

                if do2:
                    gs = s2p.tile([100, 32], BF16, tag="gs", name="gs")
                    nc.vector.tensor_tensor(out=gs[:], in0=sg[:], in1=st_f,
                                            op=OP.mult)
                    nc.tensor.matmul(out=Tt[:], lhsT=wts_sb[:],
                                     rhs=gs[:], start=False, stop=True,
                                     skip_group_check=True)
                    # hidden-window tail prep on gpsimd
                    m2 = s2p.tile([100, 32], F32, tag="m2", name="m2")
                    nc.gpsimd.tensor_tensor(
                        out=m2[:], in0=zbc[:, tl2 * BS:(tl2 + 1) * BS],
                        in1=st_f, op=OP.mult)
                    dd = s2p.tile([100, 32], F32, tag="dd", name="dd")
                    nc.gpsimd.tensor_tensor(out=dd[:], in0=st_f, in1=m2[:],
                                            op=OP.subtract)

                # --- tanhs ---
                if do1:
                    ng = s1p.tile([100, 32], BF16, tag="ng", name="ng")
                    nc.scalar.activation(out=ng[:], in_=t2[:], func=AF.Tanh)
                if do2:
                    ns = s2p.tile([100, 32], BF16, tag="ns", name="ns")
                    nc.scalar.activation(out=ns[:], in_=Tt[:],
                                         func=AF.Tanh)

                # --- tails ---
                h_new = h_prev
                st_out = st_in
                if do1:
                    # h_new = (1-z)*ng + z*h, with both factors precomputed
                    u_t = s1p.tile([100, 32], F32, tag="u", name="u")
                    nc.vector.tensor_tensor(out=u_t[:], in0=ng[:], in1=cm_t[:],
                                            op=OP.mult)
                    h_new = R[:, tl * BS:(tl + 1) * BS]
                    nc.vector.tensor_tensor(out=h_new, in0=u_t[:], in1=a_t[:],
                                            op=OP.add)
                if do2:
                    m1 = s2p.tile([100, 32], F32, tag="m1", name="m1")
                    nc.vector.tensor_tensor(
                        out=m1[:], in0=zbc[:, tl2 * BS:(tl2 + 1) * BS],
                        in1=ns[:], op=OP.mult)
                    stf2 = s2p.tile([100, BS], F32, tag="st", name="st")
                    nc.vector.tensor_tensor(out=stf2[:], in0=dd[:], in1=m1[:],
                                            op=OP.add)
                    stb2 = s2p.tile([100, BS], BF16, tag="stb", name="stb")
                    nc.vector.tensor_copy(out=stb2[:], in_=stf2[:])
                    st_out = (stf2[:], stb2[:])
                return h_new, st_out

            def emit_bulk_pieces(c):
                """Work-queue pieces for per-chunk bulk: ti, li, and the
                cosine-attention z weights. Norm chain packs 1024 slots as
                [8,128] (partition-parallel on DVE) via DMA respread; rsqrt is
                the fast-inverse-sqrt bit trick on DVE so the ACT engine never
                leaves the sigmoid/tanh table set (a Sqrt costs ~5.3us of
                table reloads per chunk). z pieces come first so the next
                chunk's scan2 unblocks earliest."""
                R = rnncs[c]
                zbc = zbp.tile([100, TCH * BS], BF16, tag="zbc", name="zbc")
                zbcs[c] = zbc
                sq = bkp.tile([100, TCH * BS], BF16, tag="sq", name="sq")
                s12 = bkp.tile([1, 2048], F32, tag="s12", name="s12")
                s1w = bkp.tile([8, 128], F32, tag="s1w", name="s1w")
                s2w = bkp.tile([8, 128], F32, tag="s2w", name="s2w")
                z_row = bkp.tile([1, TCH * BS], BF16, tag="zrow", name="zrow")
                ps = []

                def sums(hh):
                    sl = slice(hh * 512, (hh + 1) * 512)
                    rows = slice(hh * 4, (hh + 1) * 4)
                    for q in range(2):
                        qsl = slice(hh * 512 + q * 256, hh * 512 + (q + 1) * 256)
                        nc.vector.tensor_tensor(out=sq[:, qsl], in0=R[:, qsl],
                                                in1=R[:, qsl], op=OP.mult)
                    ps1 = pts.tile([1, 512], F32, space="PSUM", tag="pts",
                                  name="ps1")
                    nc.tensor.matmul(out=ps1[:], lhsT=ones_col_bf[:],
                                     rhs=R[:, sl], start=True, stop=True)
                    nc.vector.tensor_copy(out=s12[0:1, sl], in_=ps1[:])
                    nc.sync.dma_start(out=s1w[rows, :], in_=s12[0:1, sl])

                def sums2(hh):
                    sl = slice(hh * 512, (hh + 1) * 512)
                    rows = slice(hh * 4, (hh + 1) * 4)
                    ps2 = pts.tile([1, 512], F32, space="PSUM", tag="pts",
                                  name="ps2")
                    nc.tensor.matmul(out=ps2[:], lhsT=ones_col_bf[:],
                                     rhs=sq[:, sl], start=True, stop=True)
                    nc.scalar.copy(out=s12[0:1, 1024 + hh * 512:1024 + (hh + 1) * 512],
                                   in_=ps2[:])
                    nc.sync.dma_start(
                        out=s2w[rows, :],
                        in_=s12[0:1, 1024 + hh * 512:1024 + (hh + 1) * 512])

                ps.append(lambda: sums(0))
                ps.append(lambda: sums2(0))
                ps.append(lambda: sums(1))
                ps.append(lambda: sums2(1))

                def rsq_seed():
                    shi = bkp.tile([8, 128], I32, tag="shi", name="shi")
                    nc.vector.tensor_scalar(
                        out=shi[:], in0=s2w[:].bitcast(I32),
                        scalar1=1, scalar2=None, op0=OP.logical_shift_right)
                    sei = bkp.tile([8, 128], I32, tag="sei", name="sei")
                    nc.vector.tensor_scalar(
                        out=sei[:], in0=shi[:], scalar1=-1,
                        scalar2=0x5F3759DF, op0=OP.mult, op1=OP.add)
                    bulk_y[c] = sei[:].bitcast(F32)

                def rsq_nr(it):
                    y = bulk_y[c]
                    q1 = bkp.tile([8, 128], F32, tag=f"q1{it}", name="q1")
                    nc.vector.tensor_tensor(out=q1[:], in0=y, in1=y,
                                            op=OP.mult)
                    q2 = bkp.tile([8, 128], F32, tag=f"q2{it}", name="q2")
                    nc.vector.tensor_tensor(out=q2[:], in0=q1[:],
                                            in1=s2w[:], op=OP.mult)
                    q3 = bkp.tile([8, 128], F32, tag=f"q3{it}", name="q3")
                    nc.vector.tensor_scalar(
                        out=q3[:], in0=q2[:], scalar1=-0.5,
                        scalar2=1.5, op0=OP.mult, op1=OP.add)
                    yn = bkp.tile([8, 128], F32, tag=f"yn{it}", name="yn")
                    nc.vector.tensor_tensor(out=yn[:], in0=y,
                                            in1=q3[:], op=OP.mult)
                    bulk_y[c] = yn[:]

                def zfinish():
                    y = bulk_y[c]
                    att = bkp.tile([8, 128], F32, tag="att", name="att")
                    nc.vector.tensor_tensor(out=att[:], in0=s1w[:],
                                            in1=y, op=OP.mult)
                    z1 = bkp.tile([8, 128], F32, tag="z1", name="z1")
                    nc.vector.tensor_scalar(out=z1[:], in0=att[:],
                                            scalar1=0.0, scalar2=1e-3,
                                            op0=OP.max, op1=OP.mult)
                    cmp = bkp.tile([8, 128], F32, tag="cmp", name="cmp")
                    nc.vector.scalar_tensor_tensor(
                        out=cmp[:], in0=tvals_sb[:],
                        scalar=float(c * TCH), in1=lens_sb[:],
                        op0=OP.add, op1=OP.is_lt)
                    zw = bkp.tile([8, 128], BF16, tag="zw", name="zw")
                    nc.vector.tensor_tensor(out=zw[:], in0=z1[:],
                                            in1=cmp[:], op=OP.mult)
                    nc.sync.dma_start(out=z_row[0:1, :], in_=zw[:])

                ps.append(rsq_seed)
                ps.append(lambda: rsq_nr(0))
                ps.append(lambda: rsq_nr(1))
                ps.append(zfinish)

                def zbcast(hh):
                    sl = slice(hh * 512, (hh + 1) * 512)
                    pz = pts.tile([100, 512], F32, space="PSUM", tag="pts",
                                 name="pz")
                    nc.tensor.matmul(out=pz[:], lhsT=ones100[:],
                                     rhs=z_row[0:1, sl], start=True, stop=True)
                    for q in range(2):
                        nc.vector.tensor_copy(
                            out=zbc[:, hh * 512 + q * 256:hh * 512 + (q + 1) * 256],
                            in_=pz[:, q * 256:(q + 1) * 256])

                def ti_piece(hh):
                    pt = pts.tile([100, 512], F32, space="PSUM", tag="pts",
                                 name="pt")
                    nc.tensor.matmul(out=pt[:], lhsT=wti_sb[:],
                                     rhs=R[:, hh * 512:(hh + 1) * 512],
                                     start=True, stop=True)
                    for q in range(2):
                        nc.vector.tensor_scalar_add(
                            out=tis[c][:, hh * 512 + q * 256:hh * 512 + (q + 1) * 256],
                            in0=pt[:, q * 256:(q + 1) * 256],
                            scalar1=bti_sb[:])

                def li_piece(hh):
                    pl = pts.tile([1, 512], F32, space="PSUM", tag="pts",
                                 name="pl")
                    nc.tensor.matmul(out=pl[:], lhsT=wli_sb[:],
                                     rhs=R[:, hh * 512:(hh + 1) * 512],
                                     start=True, stop=True)
                    for q in range(2):
                        nc.scalar.activation(
                            out=li_row[0:1, c * 1024 + hh * 512 + q * 256:
                                       c * 1024 + hh * 512 + (q + 1) * 256],
                            in_=pl[:, q * 256:(q + 1) * 256],
                            func=AF.Identity, bias=blgr_sb[:])

                ps.append(lambda: zbcast(0))
                ps.append(lambda: ti_piece(0))
                ps.append(lambda: li_piece(0))
                ps.append(lambda: zbcast(1))
                ps.append(lambda: ti_piece(1))
                ps.append(lambda: li_piece(1))
                return ps

            # ---------- emit pipeline (interleaved) ----------
            # scan1 runs at slot S; scan2 trails by LAG slots (one chunk plus
            # 12 slots of headroom so the spread bulk pieces finish in time).
            # Gather/xp/bulk work drains from a FIFO a few pieces per slot so
            # no engine queue ever sees a long burst ahead of chain ops.
            h_prev = h0[:]
            st_cur = (st0[:], st0b[:])
            for piece in gather_pieces(0):
                piece()
            workq = []
            workq.extend(gather_pieces(1))
            LAG = TCH + 12
            TOT = NCH * TCH
            for S in range(TOT + LAG):
                if S < TOT:
                    c1, tl = divmod(S, TCH)
                    if tl == 0 and c1 + 2 < NCH:
                        workq.extend(gather_pieces(c1 + 2))
                else:
                    c1, tl = None, None
                u = S - LAG
                if u >= 0:
                    c2, tl2 = divmod(u, TCH)
                else:
                    c2, tl2 = None, None
                h_prev, st_cur = emit_pair(c1, tl, h_prev, st_cur, c2, tl2)
                if c1 is not None and tl == TCH - 1:
                    workq.extend(emit_bulk_pieces(c1))
                npop = 3 if (c1 is None or len(workq) > 20) else 2
                for _ in range(min(npop, len(workq))):
                    workq.pop(0)()

            # ---------- output ----------
            po = pg.tile([100, 32], F32, space="PSUM", tag="pg", name="po")
            nc.tensor.matmul(out=po[0:NCLS, 0:BS], lhsT=wout_sb[:],
                             rhs=st_cur[0], start=True, stop=True)
            osb = s2p.tile([NCLS, BS], F32, tag="osb", name="osb")
            nc.scalar.activation(out=osb[:], in_=po[0:NCLS, 0:BS],
                                 func=AF.Identity, bias=bout_sb[:])
            nc.sync.dma_start(out=out_d[:], in_=osb[:])

    _split_excess_waits(nc)
    return nc


_NC = None


def _get_nc():
    global _NC
    if _NC is None:
        _NC = build_graph()
    return _NC


def _prep_core_inputs(txt_s, lens_s, shared):
    """Per-core host prep: gather indices + lens."""
    flat = np.ascontiguousarray(txt_s.T).reshape(-1)  # slot k = t*BS + b
    idx_p = np.ascontiguousarray(
        flat.reshape(GT, 128).T).astype(np.int32)      # [p, tile]
    lens_p = lens_s.astype(np.float32).reshape(1, BS)
    m = dict(shared)
    m["idx"] = idx_p
    m["lens"] = lens_p
    return m


def _prep_shared(emb, W_ih, W_hh, b_ih, b_hh, W_lgr, b_lgr, W_ts, b_ts,
                 W_ti, b_ti, W_out, b_out):
    f32 = np.float32
    emb = np.ascontiguousarray(emb, dtype=f32)
    Wg = np.asarray(W_ih, f32).reshape(3, H, E)        # [g, m, e]
    arr = Wg.transpose(2, 0, 1)                        # [e, g, m]
    wih_p = np.ascontiguousarray(
        np.stack([arr[0:100], arr[100:200]], axis=1).reshape(100, 600)
    ).astype(BF)
    Whg = np.asarray(W_hh, f32).reshape(3, H, H)       # [g, m, h]
    whh_p = np.ascontiguousarray(
        Whg.transpose(2, 0, 1).reshape(H, 300)).astype(BF)
    b_ih = np.asarray(b_ih, f32)
    b_hh = np.asarray(b_hh, f32)
    biases_p = np.stack(
        [b_ih[0:H] + b_hh[0:H], b_ih[H:2 * H] + b_hh[H:2 * H], b_ih[2 * H:]],
        axis=1).astype(f32)                            # (100, 3)
    bhhn_p = b_hh[2 * H:].reshape(H, 1).astype(f32)
    wti_p = np.ascontiguousarray(np.asarray(W_ti, f32).T).astype(BF)
    bti_p = (np.asarray(b_ti, f32) + np.asarray(b_ts, f32)).reshape(H, 1)
    W_lgr = np.asarray(W_lgr, f32)
    wls_p = np.ascontiguousarray(np.repeat(W_lgr[0, H:].reshape(H, 1), H, axis=1)).astype(BF)
    wli_p = np.ascontiguousarray(W_lgr[0, :H].reshape(H, 1)).astype(BF)
    blgr_p = np.asarray(b_lgr, f32).reshape(1, 1)
    wts_p = np.ascontiguousarray(np.asarray(W_ts, f32).T).astype(BF)
    wout_p = np.ascontiguousarray(np.asarray(W_out, f32).T).astype(f32)
    bout_p = np.asarray(b_out, f32).reshape(NCLS, 1)
    tv = (np.arange(8)[:, None] * 4 + np.arange(128)[None, :] // 32).astype(f32)
    return {
        "emb": emb, "wih": wih_p, "whh": whh_p, "biases": biases_p,
        "bhhn": bhhn_p, "wti": wti_p, "bti": bti_p, "wls": wls_p,
        "wli": wli_p, "blgr": blgr_p, "wts": wts_p, "wout": wout_p,
        "bout": bout_p, "tvals": tv,
    }


def run(inputs, trace=False):
    txt = np.asarray(inputs["txt"]).astype(np.int32)
    lens = np.asarray(inputs["lens"]).astype(np.int32)
    shared = _prep_shared(
        inputs["emb"], inputs["W_ih"], inputs["W_hh"], inputs["b_ih"],
        inputs["b_hh"], inputs["W_lgr"], inputs["b_lgr"], inputs["W_ts"],
        inputs["b_ts"], inputs["W_ti"], inputs["b_ti"], inputs["W_out"],
        inputs["b_out"])
    in_maps = []
    for core in range(NCORES):
        sl = slice(core * BS, (core + 1) * BS)
        in_maps.append(_prep_core_inputs(txt[sl], lens[sl], shared))
    nc = _get_nc()
    res = run_bass_kernel_spmd(nc, in_maps, core_ids=list(range(NCORES)),
                               trace=trace)
    out = np.empty((B, NCLS), np.float32)
    for core in range(NCORES):
        out[core * BS:(core + 1) * BS] = res.results[core]["out"].T
    return out, res.exec_time_ns


def kernel(**inputs) -> np.ndarray:
    out, _ = run(inputs, trace=False)
    return out



# revision 34
# speedup vs baseline: 1.1913x; 1.0111x over previous
"""Trainium2 Bass kernel for nn_ACWAN (embedding + GRU + cosine-attention +
gated-state recurrence + output projection), data-parallel over batch on 8
NeuronCores.

Self-contained: hardcodes all shapes; host side only reformats weights/indices
and gathers per-core outputs.
"""
import sys

sys.path.insert(0, "/opt/trn_rl_repo")

import numpy as np
import ml_dtypes

import concourse.bass as bass
import concourse.mybir as mybir
from concourse.tile import TileContext
from concourse.bass_utils import run_bass_kernel_spmd
from concourse.masks import make_identity

# ---- problem dims (hardcoded) ----
B, T, E, H, NT, NCLS = 256, 512, 200, 100, 100000, 5
NCORES = 8
BS = B // NCORES            # 32 batch rows per core
TCH = 32                    # timesteps per chunk
NCH = T // TCH              # 16 chunks
SLOTS = BS * T              # 16384 gather slots per core (k = t*BS + b)
GT = SLOTS // 128           # 128 gather tiles (128 rows each)
GPC = GT // NCH             # 8 gather tiles per chunk

F32 = mybir.dt.float32
BF16 = mybir.dt.bfloat16
I32 = mybir.dt.int32
BF = ml_dtypes.bfloat16
AF = mybir.ActivationFunctionType
OP = mybir.AluOpType

MAX_WAITS_DEFAULT = 1


def _split_excess_waits(nc):
    """walrus here accepts very few sem-waits per instruction; hoist extras
    onto NoOps (1 wait each) placed just before, on the same engine."""
    n_fix = 0
    for f in nc.m.functions:
        for bb in f.blocks:
            out = []
            changed = False
            for ins in bb.instructions:
                si = ins.sync_info
                limit = MAX_WAITS_DEFAULT
                if si is not None and si.on_wait and len(si.on_wait) > limit:
                    waits = list(si.on_wait)
                    extra, keep = waits[:-limit], waits[-limit:]
                    for k, w in enumerate(extra):
                        out.append(
                            mybir.InstNoOp(
                                name=f"{ins.name}-wsplit{k}",
                                sync_info=mybir.SyncInfo(on_wait=[w], on_update=[]),
                                bass_nofuse=True,
                                engine=ins.engine,
                            )
                        )
                    ins.sync_info = mybir.SyncInfo(
                        on_wait=keep, on_update=list(si.on_update)
                    )
                    n_fix += 1
                    changed = True
                out.append(ins)
            if changed:
                bb.instructions = out
    return n_fix


def build_graph():
    nc = bass.Bass()

    dp = nc.declare_dram_parameter
    emb = dp("emb", [NT, E], F32, isOutput=False)
    idx = dp("idx", [128, GT], I32, isOutput=False)          # [p, tile]
    wih = dp("wih", [100, 600], BF16, isOutput=False)        # [e, (chunk,gate,m)]
    whh = dp("whh", [100, 300], BF16, isOutput=False)        # [h, (gate,m)]
    biases = dp("biases", [100, 3], F32, isOutput=False)     # r,z fused; n = b_ih_n
    bhhn = dp("bhhn", [100, 1], F32, isOutput=False)         # b_hh n-gate column
    wti = dp("wti", [100, 100], BF16, isOutput=False)
    bti = dp("bti", [100, 1], F32, isOutput=False)           # b_ti + b_ts fused
    wls = dp("wls", [100, 100], BF16, isOutput=False)
    wli = dp("wli", [100, 1], BF16, isOutput=False)
    blgr = dp("blgr", [1, 1], F32, isOutput=False)
    wts = dp("wts", [100, 100], BF16, isOutput=False)
    wout = dp("wout", [100, NCLS], F32, isOutput=False)
    bout = dp("bout", [NCLS, 1], F32, isOutput=False)
    lens = dp("lens", [1, BS], F32, isOutput=False)
    tvals = dp("tvals", [8, 128], F32, isOutput=False)       # p*4 + f//32
    out_d = dp("out", [NCLS, BS], F32, isOutput=True)

    with TileContext(nc) as tc:
        with tc.tile_pool(name="const", bufs=1) as cp, \
             tc.tile_pool(name="big", bufs=1) as bigp, \
             tc.tile_pool(name="xpring", bufs=4) as xpp, \
             tc.tile_pool(name="rnnring", bufs=3) as rnp, \
             tc.tile_pool(name="zbring", bufs=3) as zbp, \
             tc.tile_pool(name="gring", bufs=6) as gp, \
             tc.tile_pool(name="rtring", bufs=3) as rtp, \
             tc.tile_pool(name="s1sc", bufs=3) as s1p, \
             tc.tile_pool(name="s2sc", bufs=3) as s2p, \
             tc.tile_pool(name="blksc", bufs=1) as bkp, \
             tc.tile_pool(name="pa", bufs=2, space="PSUM") as pa, \
             tc.tile_pool(name="p1", bufs=2, space="PSUM") as p1, \
             tc.tile_pool(name="pg", bufs=2, space="PSUM") as pg, \
             tc.tile_pool(name="pts", bufs=2, space="PSUM") as pts:

            # ---------- load constants ----------
            idx_sb = cp.tile([128, GT], I32)
            nc.sync.dma_start(out=idx_sb[:], in_=idx[:])
            wih_sb = cp.tile([100, 600], BF16)
            nc.sync.dma_start(out=wih_sb[:], in_=wih[:])
            whh_sb = cp.tile([100, 300], BF16)
            nc.sync.dma_start(out=whh_sb[:], in_=whh[:])
            bias_sb = cp.tile([100, 3], F32)
            nc.sync.dma_start(out=bias_sb[:], in_=biases[:])
            bhhn_sb = cp.tile([100, 1], F32)
            nc.sync.dma_start(out=bhhn_sb[:], in_=bhhn[:])
            wti_sb = cp.tile([100, 100], BF16)
            nc.sync.dma_start(out=wti_sb[:], in_=wti[:])
            bti_sb = cp.tile([100, 1], F32)
            nc.sync.dma_start(out=bti_sb[:], in_=bti[:])
            wlsr_sb = cp.tile([100, 100], BF16)
            nc.sync.dma_start(out=wlsr_sb[:], in_=wls[:])
            wli_sb = cp.tile([100, 1], BF16)
            nc.sync.dma_start(out=wli_sb[:], in_=wli[:])
            blgr_sb = cp.tile([1, 1], F32)
            nc.sync.dma_start(out=blgr_sb[:], in_=blgr[:])
            wts_sb = cp.tile([100, 100], BF16)
            nc.sync.dma_start(out=wts_sb[:], in_=wts[:])
            wout_sb = cp.tile([100, NCLS], F32)
            nc.sync.dma_start(out=wout_sb[:], in_=wout[:])
            bout_sb = cp.tile([NCLS, 1], F32)
            nc.sync.dma_start(out=bout_sb[:], in_=bout[:])
            tvals_sb = cp.tile([8, 128], F32)
            nc.sync.dma_start(out=tvals_sb[:], in_=tvals[:])

            lens_sb = cp.tile([8, 128], F32)
            lens_bcast = bass.AP(
                tensor=lens[:].tensor, offset=0,
                ap=[[0, 8], [0, 4], [1, BS]],
            )
            nc.sync.dma_start(
                out=lens_sb[:].rearrange("p (a b) -> p a b", a=4),
                in_=lens_bcast)

            ident = cp.tile([128, 128], BF16)
            make_identity(nc, ident[:])
            ones100 = cp.tile([1, 100], BF16)
            nc.vector.memset(ones100[:], 1.0)
            ones_col_bf = cp.tile([100, 1], BF16)
            nc.vector.memset(ones_col_bf[:], 1.0)
            h0 = cp.tile([100, BS], BF16)
            nc.vector.memset(h0[:], 0.0)
            st0 = cp.tile([100, BS], F32)
            nc.vector.memset(st0[:], 0.0)
            st0b = cp.tile([100, BS], BF16)
            nc.vector.memset(st0b[:], 0.0)

            # persistent rows
            li_row = bigp.tile([1, SLOTS], BF16)
            tis = [bigp.tile([100, TCH * BS], BF16, tag=f"ti{c}", name=f"ti{c}")
                   for c in range(NCH)]

            xpcs = {}
            rnncs = {}
            zbcs = {}
            bulk_y = {}

            # ---------- stage emitters ----------
            gbigs = {}

            gtiles = {}
            rttiles = {}

            def gather_a(c, j):
                """Indirect gather of tile j of chunk c (gpsimd SWDGE)."""
                g = gp.tile([128, E], BF16, tag="g", name="g")
                gtiles[(c, j)] = g
                tile_id = c * GPC + j
                nc.gpsimd.indirect_dma_start(
                    out=g[:], out_offset=None, in_=emb[:],
                    in_offset=bass.IndirectOffsetOnAxis(
                        ap=idx_sb[:, tile_id:tile_id + 1], axis=0),
                )

            def gather_b(c, j):
                """Transpose gathered rows into [E, slots] layout."""
                g = gtiles[(c, j)]
                rt = rtp.tile([100, 256], BF16, tag="rt", name="rt")
                rttiles[(c, j)] = rt
                for ch in range(2):
                    tr = pa.tile([100, 384], BF16, space="PSUM", tag="pa",
                                 name="tr")
                    nc.tensor.transpose(
                        out=tr[:, 0:128],
                        in_=g[:, ch * 100:(ch + 1) * 100],
                        identity=ident[:])
                    if ch == 0:
                        nc.vector.tensor_copy(out=rt[:, 0:128], in_=tr[:, 0:128])
                    else:
                        nc.scalar.copy(out=rt[:, 128:256], in_=tr[:, 0:128])

            def gather_c(c, j, gate):
                """xp matmul for one gate of tile j."""
                xprz, xpn = xpcs[c]
                rt = rttiles[(c, j)]
                xg = pa.tile([100, 384], F32, space="PSUM", tag="pa",
                             name="xg")
                nc.tensor.matmul(
                    out=xg[:, 0:128], lhsT=wih_sb[:, (0 * 3 + gate) * 100:(0 * 3 + gate) * 100 + 100],
                    rhs=rt[:, 0:128], start=True, stop=False)
                nc.tensor.matmul(
                    out=xg[:, 0:128], lhsT=wih_sb[:, (1 * 3 + gate) * 100:(1 * 3 + gate) * 100 + 100],
                    rhs=rt[:, 128:256], start=False, stop=True)
                if gate < 2:
                    dst = xprz[:, gate * 1024 + j * 128:gate * 1024 + (j + 1) * 128]
                    nc.vector.tensor_scalar_add(
                        out=dst, in0=xg[:, 0:128], scalar1=bias_sb[:, gate:gate + 1])
                else:
                    nc.scalar.activation(
                        out=xpn[:, j * 128:(j + 1) * 128], in_=xg[:, 0:128],
                        func=AF.Identity, bias=bias_sb[:, 2:3])

            def gather_pieces(c):
                """Work-queue pieces to prepare chunk c's xp (emitted spread
                across slots: the 1.1us SWDGE op must not head-of-line-block
                the per-step gpsimd tail ops)."""
                xpcs[c] = (
                    xpp.tile([100, 2 * TCH * BS], BF16, tag="xprz",
                             name="xprz"),
                    xpp.tile([100, TCH * BS], BF16, tag="xpn", name="xpn"),
                )
                ps = []
                for j in range(GPC):
                    ps.append(lambda c=c, j=j: gather_a(c, j))
                    ps.append(lambda c=c, j=j: gather_b(c, j))
                    ps.append(lambda c=c, j=j: gather_c(c, j, 0))
                    ps.append(lambda c=c, j=j: (gather_c(c, j, 1),
                                                gather_c(c, j, 2)))
                return ps

            def emit_pair(c1, tl, h_prev, st_in, c2, tl2):
                """Interleaved scan1 step (c1, tl) + scan2 step (c2, tl2).
                c1 None => scan1 done; c2 None => scan2 not started/active.
                st_in = (st_f32, st_bf16) APs; the bf16 cast for the next
                gate matmul runs on the pipelined DVE."""
                do1 = c1 is not None
                do2 = c2 is not None
                xprz = xpn = R = P = None
                if do1:
                    xprz, xpn = xpcs[c1]
                    if tl == 0:
                        rnncs[c1] = rnp.tile([100, TCH * BS], BF16, tag="rnnf",
                                             name="rnnf")
                    R = rnncs[c1]
                if do2:
                    st_f, st_b = st_in
                    zbc = zbcs[c2]

                # --- matmuls first (both scans) ---
                if do1:
                    P = p1.tile([100, 96], F32, space="PSUM", tag="p1",
                                name="P")
                    nc.vector.tensor_copy(
                        out=P[:, 0:64],
                        in_=xprz[:].rearrange("p (g r) -> p g r", g=2)[:, :, tl * BS:(tl + 1) * BS])
                    nc.tensor.matmul(out=P[:, 0:32], lhsT=whh_sb[:, 0:100],
                                     rhs=h_prev, start=False, stop=True,
                                     skip_group_check=True)
                    nc.tensor.matmul(out=P[:, 32:64], lhsT=whh_sb[:, 100:200],
                                     rhs=h_prev, start=False, stop=True,
                                     skip_group_check=True)
                    nc.tensor.matmul(out=P[:, 64:96], lhsT=whh_sb[:, 200:300],
                                     rhs=h_prev, start=True, stop=True,
                                     skip_group_check=True)
                if do2:
                    t = c2 * TCH + tl2
                    Tg = pg.tile([100, 32], F32, space="PSUM", tag="pg",
                                 name="Tg")
                    Tt = pg.tile([100, 32], F32, space="PSUM", tag="pg",
                                 name="Tt")
                    nc.tensor.matmul(
                        out=Tg[:], lhsT=ones100[:],
                        rhs=li_row[0:1, t * BS:(t + 1) * BS],
                        start=True, stop=False)
                    nc.tensor.matmul(out=Tg[:], lhsT=wlsr_sb[:],
                                     rhs=st_b, start=False, stop=True,
                                     skip_group_check=True)
                    nc.vector.tensor_copy(
                        out=Tt[:], in_=tis[c2][:, tl2 * BS:(tl2 + 1) * BS])

                # --- sigmoids ---
                if do1:
                    S = s1p.tile([100, 64], BF16, tag="S", name="S")
                    nc.scalar.activation(out=S[:], in_=P[:, 0:64],
                                         func=AF.Sigmoid)
                if do2:
                    sg = s2p.tile([100, 32], BF16, tag="sg", name="sg")
                    nc.scalar.activation(out=sg[:], in_=Tg[:],
                                         func=AF.Sigmoid)

                # --- middles ---
                if do1:
                    # (P_n + b_hh_n) * r, bias folded in via per-partition STT
                    t1 = s1p.tile([100, 32], F32, tag="t1", name="t1")
                    nc.vector.scalar_tensor_tensor(
                        out=t1[:], in0=P[:, 64:96], scalar=bhhn_sb[:],
                        in1=S[:, 0:32], op0=OP.add, op1=OP.mult)
                    t2 = s1p.tile([100, 32], F32, tag="t2", name="t2")
                    nc.vector.tensor_tensor(out=t2[:], in0=t1[:],
                                            in1=xpn[:, tl * BS:(tl + 1) * BS],
                                            op=OP.add)
                    # tail prep off the tanh chain: a = z*h, cm = 1-z
                    # (on DVE: gpsimd runs the 1.1us SWDGE gathers, which
                    # would head-of-line-block these chain-adjacent ops)
                    a_t = s1p.tile([100, 32], F32, tag="a", name="a")
                    nc.vector.tensor_tensor(out=a_t[:], in0=S[:, 32:64],
                                            in1=h_prev, op=OP.mult)
                    cm_t = s1p.tile([100, 32], F32, tag="cm", name="cm")
                    nc.vector.tensor_scalar(
                        out=cm_t[:], in0=S[:, 32:64], scalar1=-1.0,
                        scalar2=1.0, op0=OP.mult, op1=OP.add)

                if do2:
                    gs = s2p.tile([100, 32], BF16, tag="gs", name="gs")
                    nc.vector.tensor_tensor(out=gs[:], in0=sg[:], in1=st_f,
                                            op=OP.mult)
                    nc.tensor.matmul(out=Tt[:], lhsT=wts_sb[:],
                                     rhs=gs[:], start=False, stop=True,
                                     skip_group_check=True)
                    # hidden-window tail prep on gpsimd
                    m2 = s2p.tile([100, 32], F32, tag="m2", name="m2")
                    nc.gpsimd.tensor_tensor(
                        out=m2[:], in0=zbc[:, tl2 * BS:(tl2 + 1) * BS],
                        in1=st_f, op=OP.mult)
                    dd = s2p.tile([100, 32], F32, tag="dd", name="dd")
                    nc.gpsimd.tensor_tensor(out=dd[:], in0=st_f, in1=m2[:],
                                            op=OP.subtract)

                # --- tanhs ---
                if do1:
                    ng = s1p.tile([100, 32], BF16, tag="ng", name="ng")
                    nc.scalar.activation(out=ng[:], in_=t2[:], func=AF.Tanh)
                if do2:
                    ns = s2p.tile([100, 32], BF16, tag="ns", name="ns")
                    nc.scalar.activation(out=ns[:], in_=Tt[:],
                                         func=AF.Tanh)

                # --- tails ---
                h_new = h_prev
                st_out = st_in
                if do1:
                    # h_new = (1-z)*ng + z*h, with both factors precomputed
                    u_t = s1p.tile([100, 32], F32, tag="u", name="u")
                    nc.vector.tensor_tensor(out=u_t[:], in0=ng[:], in1=cm_t[:],
                                            op=OP.mult)
                    h_new = R[:, tl * BS:(tl + 1) * BS]
                    nc.vector.tensor_tensor(out=h_new, in0=u_t[:], in1=a_t[:],
                                            op=OP.add)
                if do2:
                    m1 = s2p.tile([100, 32], F32, tag="m1", name="m1")
                    nc.vector.tensor_tensor(
                        out=m1[:], in0=zbc[:, tl2 * BS:(tl2 + 1) * BS],
                        in1=ns[:], op=OP.mult)
                    stf2 = s2p.tile([100, BS], F32, tag="st", name="st")
                    nc.vector.tensor_tensor(out=stf2[:], in0=dd[:], in1=m1[:],
                                            op=OP.add)
                    stb2 = s2p.tile([100, BS], BF16, tag="stb", name="stb")
                    nc.vector.tensor_copy(out=stb2[:], in_=stf2[:])
                    st_out = (stf2[:], stb2[:])
                return h_new, st_out

            def emit_bulk_pieces(c):
                """Work-queue pieces for per-chunk bulk: ti, li, and the
                cosine-attention z weights. Norm chain packs 1024 slots as
                [8,128] (partition-parallel on DVE) via DMA respread; rsqrt is
                the fast-inverse-sqrt bit trick on DVE so the ACT engine never
                leaves the sigmoid/tanh table set (a Sqrt costs ~5.3us of
                table reloads per chunk). z pieces come first so the next
                chunk's scan2 unblocks earliest."""
                R = rnncs[c]
                zbc = zbp.tile([100, TCH * BS], BF16, tag="zbc", name="zbc")
                zbcs[c] = zbc
                sq = bkp.tile([100, TCH * BS], BF16, tag="sq", name="sq")
                s12 = bkp.tile([1, 2048], F32, tag="s12", name="s12")
                s1w = bkp.tile([8, 128], F32, tag="s1w", name="s1w")
                s2w = bkp.tile([8, 128], F32, tag="s2w", name="s2w")
                z_row = bkp.tile([1, TCH * BS], BF16, tag="zrow", name="zrow")
                ps = []

                def sums(hh):
                    sl = slice(hh * 512, (hh + 1) * 512)
                    rows = slice(hh * 4, (hh + 1) * 4)
                    for q in range(2):
                        qsl = slice(hh * 512 + q * 256, hh * 512 + (q + 1) * 256)
                        nc.vector.tensor_tensor(out=sq[:, qsl], in0=R[:, qsl],
                                                in1=R[:, qsl], op=OP.mult)
                    ps1 = pts.tile([1, 512], F32, space="PSUM", tag="pts",
                                  name="ps1")
                    nc.tensor.matmul(out=ps1[:], lhsT=ones_col_bf[:],
                                     rhs=R[:, sl], start=True, stop=True)
                    nc.vector.tensor_copy(out=s12[0:1, sl], in_=ps1[:])
                    nc.sync.dma_start(out=s1w[rows, :], in_=s12[0:1, sl])

                def sums2(hh):
                    sl = slice(hh * 512, (hh + 1) * 512)
                    rows = slice(hh * 4, (hh + 1) * 4)
                    ps2 = pts.tile([1, 512], F32, space="PSUM", tag="pts",
                                  name="ps2")
                    nc.tensor.matmul(out=ps2[:], lhsT=ones_col_bf[:],
                                     rhs=sq[:, sl], start=True, stop=True)
                    nc.scalar.copy(out=s12[0:1, 1024 + hh * 512:1024 + (hh + 1) * 512],
                                   in_=ps2[:])
                    nc.sync.dma_start(
                        out=s2w[rows, :],
                        in_=s12[0:1, 1024 + hh * 512:1024 + (hh + 1) * 512])

                ps.append(lambda: sums(0))
                ps.append(lambda: sums2(0))
                ps.append(lambda: sums(1))
                ps.append(lambda: sums2(1))

                def rsq_seed():
                    shi = bkp.tile([8, 128], I32, tag="shi", name="shi")
                    nc.vector.tensor_scalar(
                        out=shi[:], in0=s2w[:].bitcast(I32),
                        scalar1=1, scalar2=None, op0=OP.logical_shift_right)
                    sei = bkp.tile([8, 128], I32, tag="sei", name="sei")
                    nc.vector.tensor_scalar(
                        out=sei[:], in0=shi[:], scalar1=-1,
                        scalar2=0x5F3759DF, op0=OP.mult, op1=OP.add)
                    bulk_y[c] = sei[:].bitcast(F32)

                def rsq_nr(it):
                    y = bulk_y[c]
                    q1 = bkp.tile([8, 128], F32, tag=f"q1{it}", name="q1")
                    nc.vector.tensor_tensor(out=q1[:], in0=y, in1=y,
                                            op=OP.mult)
                    q2 = bkp.tile([8, 128], F32, tag=f"q2{it}", name="q2")
                    nc.vector.tensor_tensor(out=q2[:], in0=q1[:],
                                            in1=s2w[:], op=OP.mult)
                    q3 = bkp.tile([8, 128], F32, tag=f"q3{it}", name="q3")
                    nc.vector.tensor_scalar(
                        out=q3[:], in0=q2[:], scalar1=-0.5,
                        scalar2=1.5, op0=OP.mult, op1=OP.add)
                    yn = bkp.tile([8, 128], F32, tag=f"yn{it}", name="yn")
                    nc.vector.tensor_tensor(out=yn[:], in0=y,
                                            in1=q3[:], op=OP.mult)
                    bulk_y[c] = yn[:]

                def zfinish():
                    y = bulk_y[c]
                    att = bkp.tile([8, 128], F32, tag="att", name="att")
                    nc.vector.tensor_tensor(out=att[:], in0=s1w[:],
                                            in1=y, op=OP.mult)
                    z1 = bkp.tile([8, 128], F32, tag="z1", name="z1")
                    nc.vector.tensor_scalar(out=z1[:], in0=att[:],
                                            scalar1=0.0, scalar2=1e-3,
                                            op0=OP.max, op1=OP.mult)
                    cmp = bkp.tile([8, 128], F32, tag="cmp", name="cmp")
                    nc.vector.scalar_tensor_tensor(
                        out=cmp[:], in0=tvals_sb[:],
                        scalar=float(c * TCH), in1=lens_sb[:],
                        op0=OP.add, op1=OP.is_lt)
                    zw = bkp.tile([8, 128], BF16, tag="zw", name="zw")
                    nc.vector.tensor_tensor(out=zw[:], in0=z1[:],
                                            in1=cmp[:], op=OP.mult)
                    nc.sync.dma_start(out=z_row[0:1, :], in_=zw[:])

                ps.append(rsq_seed)
                ps.append(lambda: rsq_nr(0))
                ps.append(lambda: rsq_nr(1))
                ps.append(zfinish)

                def zbcast(hh):
                    sl = slice(hh * 512, (hh + 1) * 512)
                    pz = pts.tile([100, 512], F32, space="PSUM", tag="pts",
                                 name="pz")
                    nc.tensor.matmul(out=pz[:], lhsT=ones100[:],
                                     rhs=z_row[0:1, sl], start=True, stop=True)
                    for q in range(2):
                        nc.vector.tensor_copy(
                            out=zbc[:, hh * 512 + q * 256:hh * 512 + (q + 1) * 256],
                            in_=pz[:, q * 256:(q + 1) * 256])

                def ti_piece(hh):
                    pt = pts.tile([100, 512], F32, space="PSUM", tag="pts",
                                 name="pt")
                    nc.tensor.matmul(out=pt[:], lhsT=wti_sb[:],
                                     rhs=R[:, hh * 512:(hh + 1) * 512],
                                     start=True, stop=True)
                    for q in range(2):
                        nc.vector.tensor_scalar_add(
                            out=tis[c][:, hh * 512 + q * 256:hh * 512 + (q + 1) * 256],
                            in0=pt[:, q * 256:(q + 1) * 256],
                            scalar1=bti_sb[:])

                def li_piece(hh):
                    pl = pts.tile([1, 512], F32, space="PSUM", tag="pts",
                                 name="pl")
                    nc.tensor.matmul(out=pl[:], lhsT=wli_sb[:],
                                     rhs=R[:, hh * 512:(hh + 1) * 512],
                                     start=True, stop=True)
                    for q in range(2):
                        nc.scalar.activation(
                            out=li_row[0:1, c * 1024 + hh * 512 + q * 256:
                                       c * 1024 + hh * 512 + (q + 1) * 256],
                            in_=pl[:, q * 256:(q + 1) * 256],
                            func=AF.Identity, bias=blgr_sb[:])

                ps.append(lambda: zbcast(0))
                ps.append(lambda: ti_piece(0))
                ps.append(lambda: li_piece(0))
                ps.append(lambda: zbcast(1))
                ps.append(lambda: ti_piece(1))
                ps.append(lambda: li_piece(1))
                return ps

            # ---------- emit pipeline (interleaved) ----------
            # scan1 runs at slot S; scan2 trails by LAG slots (one chunk plus
            # 12 slots of headroom so the spread bulk pieces finish in time).
            # Gather/xp/bulk work drains from a FIFO a few pieces per slot so
            # no engine queue ever sees a long burst ahead of chain ops.
            h_prev = h0[:]
            st_cur = (st0[:], st0b[:])
            for piece in gather_pieces(0):
                piece()
            workq = []
            workq.extend(gather_pieces(1))
            LAG = TCH + 12
            TOT = NCH * TCH
            for S in range(TOT + LAG):
                if S < TOT:
                    c1, tl = divmod(S, TCH)
                    if tl == 8 and c1 + 2 < NCH:
                        workq.extend(gather_pieces(c1 + 2))
                else:
                    c1, tl = None, None
                u = S - LAG
                if u >= 0:
                    c2, tl2 = divmod(u, TCH)
                else:
                    c2, tl2 = None, None
                h_prev, st_cur = emit_pair(c1, tl, h_prev, st_cur, c2, tl2)
                if c1 is not None and tl == TCH - 1:
                    workq.extend(emit_bulk_pieces(c1))
                npop = 3 if (c1 is None or len(workq) > 20) else 2
                for _ in range(min(npop, len(workq))):
                    workq.pop(0)()

            # ---------- output ----------
            po = pg.tile([100, 32], F32, space="PSUM", tag="pg", name="po")
            nc.tensor.matmul(out=po[0:NCLS, 0:BS], lhsT=wout_sb[:],
                             rhs=st_cur[0], start=True, stop=True)
            osb = s2p.tile([NCLS, BS], F32, tag="osb", name="osb")
            nc.scalar.activation(out=osb[:], in_=po[0:NCLS, 0:BS],
                                 func=AF.Identity, bias=bout_sb[:])
            nc.sync.dma_start(out=out_d[:], in_=osb[:])

    _split_excess_waits(nc)
    return nc


_NC = None


def _get_nc():
    global _NC
    if _NC is None:
        _NC = build_graph()
    return _NC


def _prep_core_inputs(txt_s, lens_s, shared):
    """Per-core host prep: gather indices + lens."""
    flat = np.ascontiguousarray(txt_s.T).reshape(-1)  # slot k = t*BS + b
    idx_p = np.ascontiguousarray(
        flat.reshape(GT, 128).T).astype(np.int32)      # [p, tile]
    lens_p = lens_s.astype(np.float32).reshape(1, BS)
    m = dict(shared)
    m["idx"] = idx_p
    m["lens"] = lens_p
    return m


def _prep_shared(emb, W_ih, W_hh, b_ih, b_hh, W_lgr, b_lgr, W_ts, b_ts,
                 W_ti, b_ti, W_out, b_out):
    f32 = np.float32
    emb = np.ascontiguousarray(emb, dtype=f32)
    Wg = np.asarray(W_ih, f32).reshape(3, H, E)        # [g, m, e]
    arr = Wg.transpose(2, 0, 1)                        # [e, g, m]
    wih_p = np.ascontiguousarray(
        np.stack([arr[0:100], arr[100:200]], axis=1).reshape(100, 600)
    ).astype(BF)
    Whg = np.asarray(W_hh, f32).reshape(3, H, H)       # [g, m, h]
    whh_p = np.ascontiguousarray(
        Whg.transpose(2, 0, 1).reshape(H, 300)).astype(BF)
    b_ih = np.asarray(b_ih, f32)
    b_hh = np.asarray(b_hh, f32)
    biases_p = np.stack(
        [b_ih[0:H] + b_hh[0:H], b_ih[H:2 * H] + b_hh[H:2 * H], b_ih[2 * H:]],
        axis=1).astype(f32)                            # (100, 3)
    bhhn_p = b_hh[2 * H:].reshape(H, 1).astype(f32)
    wti_p = np.ascontiguousarray(np.asarray(W_ti, f32).T).astype(BF)
    bti_p = (np.asarray(b_ti, f32) + np.asarray(b_ts, f32)).reshape(H, 1)
    W_lgr = np.asarray(W_lgr, f32)
    wls_p = np.ascontiguousarray(np.repeat(W_lgr[0, H:].reshape(H, 1), H, axis=1)).astype(BF)
    wli_p = np.ascontiguousarray(W_lgr[0, :H].reshape(H, 1)).astype(BF)
    blgr_p = np.asarray(b_lgr, f32).reshape(1, 1)
    wts_p = np.ascontiguousarray(np.asarray(W_ts, f32).T).astype(BF)
    wout_p = np.ascontiguousarray(np.asarray(W_out, f32).T).astype(f32)
    bout_p = np.asarray(b_out, f32).reshape(NCLS, 1)
    tv = (np.arange(8)[:, None] * 4 + np.arange(128)[None, :] // 32).astype(f32)
    return {
        "emb": emb, "wih": wih_p, "whh": whh_p, "biases": biases_p,
        "bhhn": bhhn_p, "wti": wti_p, "bti": bti_p, "wls": wls_p,
        "wli": wli_p, "blgr": blgr_p, "wts": wts_p, "wout": wout_p,
        "bout": bout_p, "tvals": tv,
    }


def run(inputs, trace=False):
    txt = np.asarray(inputs["txt"]).astype(np.int32)
    lens = np.asarray(inputs["lens"]).astype(np.int32)
    shared = _prep_shared(
        inputs["emb"], inputs["W_ih"], inputs["W_hh"], inputs["b_ih"],
        inputs["b_hh"], inputs["W_lgr"], inputs["b_lgr"], inputs["W_ts"],
        inputs["b_ts"], inputs["W_ti"], inputs["b_ti"], inputs["W_out"],
        inputs["b_out"])
    in_maps = []
    for core in range(NCORES):
        sl = slice(core * BS, (core + 1) * BS)
        in_maps.append(_prep_core_inputs(txt[sl], lens[sl], shared))
    nc = _get_nc()
    res = run_bass_kernel_spmd(nc, in_maps, core_ids=list(range(NCORES)),
                               trace=trace)
    out = np.empty((B, NCLS), np.float32)
    for core in range(NCORES):
        out[core * BS:(core + 1) * BS] = res.results[core]["out"].T
    return out, res.exec_time_ns


def kernel(**inputs) -> np.ndarray:
    out, _ = run(inputs, trace=False)
    return out

